# revision 27
# baseline (speedup 1.0000x reference)
"""Causal multi-head attention kernel for Trainium2 (8 NeuronCores).

Problem: B=2, H=16, S=2048, D=64 causal attention (softmax over last axis).
Sharding: 32 (batch, head) pairs split 4-per-core across 8 cores; each core
computes its heads independently (no collectives).

v3 per-core algorithm (pair-interleaved, S-transposed layout; see _emit_v3):
  - Host pre-packs qt/kt with the two heads of a pair on SBUF partitions
    0-63 / 64-127.  Their K=64 QK matmuls auto-derive PE tile_position
    (0,0)/(64,0) from base partitions, so adjacent A/B matmuls run
    CONCURRENTLY on the PE row-groups (measured exactly 2x).
  - Each 512-col fill writes [A|B] halves of one [128,1024] f32 PSUM strip
    slot (2 slots, 4 banks), and ONE joint ACTIVATE exps BOTH heads
    (FD=1024) into a per-pair P^T store in SBUF.  ACT per-instruction
    overhead is ~430 ns on HW, so instruction count dominates the exp cost;
    the joint exps cut it from 96 to 80 per core.
  - PV kb-outer: head A's 16 chains eager in 3 PSUM banks (7/7/2 + ones-col
    row-sum trick); head B's qs9-15 chains eager in the spare 8th bank;
    B's qs0-6 / qs7-8 chains reuse A's banks as A's bank-group
    normalizations release them.  Every bank group is normalized (batched
    DVE reciprocal + per-row scalar mul) and DMA'd out the moment it closes.
  - Input DMAs are batched per iteration and spread across the SP HWDGE,
    ACT HWDGE, and gpsimd SWDGE rings.

Measured per-iteration HW time (8-core SPMD, repeat-differenced): ~94 us
vs 127-134 us for the per-head sequential baseline (VERSION=2 path).

kernel(Q, K, V, mask) takes the full unsharded fp32 inputs and returns the
full [2, 16, 2048, 64] fp32 output.
"""

import sys

if "/opt/trn_rl_repo" not in sys.path:
    sys.path.insert(0, "/opt/trn_rl_repo")

import numpy as np
import ml_dtypes

B, H, S, D = 2, 16, 2048, 64
N_CORES = 8
HEADS_PER_CORE = (B * H) // N_CORES  # 4
KB = S // 128  # 16 k-blocks per head
QS = S // 128  # 16 q-subblocks per head

_BF16 = ml_dtypes.bfloat16

# accumulator bank packing: q_subs 0-6 -> bank A, 7-13 -> bank B, 14-15 -> C
_BANK_FIRST = (0, 7, 14)   # first q_sub written in each accumulator bank
_BANK_LAST = (6, 13, 15)   # last q_sub written in each accumulator bank

_built = {}
STRIP_W = 1024
MODE = "full"  # full | qk_only | qk_exp (timing ablations)
ST_BUFS = 2  # PSUM strip-tile slots  # S^T strip tile width (PSUM free elems)
PT_BUFS = 4   # SBUF P^T tile slots (exp outputs)
STAGGERED = True  # staggered_reset on the timing loop (overlaps iterations)
VERSION = 3   # 2 = per-head sequential; 3 = pair-interleaved row-tiled;
              # 4 = v3 + packed 512-col fills crossing strip boundaries
              # (v4 measured SLOWER: PE segment/LDW overhead > ACT savings)


def _emit_v3(tc, nc, mybir, qt, kt, vg, tri, o, reps=1):
    """Pair-interleaved causal attention.

    The two heads of a pair occupy SBUF partitions 0-63 (A) and 64-127 (B) of
    qt/kt.  Their QK matmuls carry tile_position (0,0)/(64,0) automatically
    (base partitions), so adjacent A/B matmuls run CONCURRENTLY on the PE's
    row-groups.  Each 512-col fill writes [A|B] side by side into one
    [128,1024] PSUM strip slot, and ONE joint ACTIVATE exps both heads into a
    per-pair P^T store in SBUF (pts).  PV for head A runs kb-outer eagerly
    (accs in 3 banks); head B's qs7-13 chains run eagerly in the spare 8th
    bank, and its qs0-6 / qs14-15 chains reuse A's banks as A's
    normalizations release them.  PSUM: 2x2-bank strip slots + 4 acc banks =
    exactly 8.
    """
    from contextlib import ExitStack, nullcontext

    f32 = mybir.dt.float32
    bf = mybir.dt.bfloat16
    Exp = mybir.ActivationFunctionType.Exp

    offs = []
    off = 0
    for kb in range(KB):
        offs.append(off)
        off += S - 128 * kb
    W = off  # 17408

    with ExitStack() as ctx:
        const = ctx.enter_context(tc.tile_pool(name="const", bufs=1))
        qk = ctx.enter_context(tc.tile_pool(name="qk", bufs=1))
        vpool = ctx.enter_context(tc.tile_pool(name="vp", bufs=1))
        ptsp = ctx.enter_context(tc.tile_pool(name="ptsp", bufs=2))
        outp = ctx.enter_context(tc.tile_pool(name="outp", bufs=2))
        small = ctx.enter_context(tc.tile_pool(name="small", bufs=4))
        stp = ctx.enter_context(tc.tile_pool(name="stp", bufs=ST_BUFS, space="PSUM"))
        accp = ctx.enter_context(tc.tile_pool(name="accp", bufs=1, space="PSUM"))

        tri2 = const.tile([128, 2, 128], bf, name="tri2")
        nc.gpsimd.dma_start(tri2[:, 0, :], tri[:, :])
        nc.gpsimd.dma_start(tri2[:, 1, :], tri[:, :])

        # dummy exp: hoists the ~2.7us ACT table load ahead of the input DMAs
        warm = const.tile([128, 1], f32, name="warm")
        nc.vector.memset(warm[:, :], 0.0)
        nc.scalar.activation(warm[:, :], warm[:, :], Exp)

        with (tc.For_i(0, reps, 1, hint_engines=(mybir.EngineType.PE,),
                       staggered_reset=STAGGERED)
              if reps > 1 else nullcontext()):
            # iteration-wide input loads, spread over SP/ACT/SWDGE rings
            qt_ts, kt_ts = [], []
            vg_t = vpool.tile([128, HEADS_PER_CORE, KB, 65], bf, tag="vg",
                              name="vg")
            for p in range(HEADS_PER_CORE // 2):
                qt_t = qk.tile([128, S], bf, tag=f"qt{p}", name=f"qt_{p}")
                kt_t = qk.tile([128, S], bf, tag=f"kt{p}", name=f"kt_{p}")
                nc.sync.dma_start(kt_t[:, :128], kt[p][:, :128])
                if p == 0:
                    nc.sync.dma_start(qt_t[:, :512], qt[p][:, :512])
                    nc.sync.dma_start(qt_t[:, 512:], qt[p][:, 512:])
                else:
                    nc.sync.dma_start(qt_t[:, :], qt[p][:, :])
                nc.scalar.dma_start(kt_t[:, 128:], kt[p][:, 128:])
                qt_ts.append(qt_t)
                kt_ts.append(kt_t)
            vg_src = vg.rearrange("h p k c -> p h k c")
            nc.gpsimd.dma_start(vg_t[:, 0:1, :, :], vg_src[:, 0:1, :, :])
            nc.gpsimd.dma_start(vg_t[:, 1:, :, :], vg_src[:, 1:, :, :])

            for p in range(HEADS_PER_CORE // 2):
                qt_t, kt_t = qt_ts[p], kt_ts[p]
                pts = ptsp.tile([128, 2, W], bf, tag="pts", name=f"pts_{p}")
                ots = [outp.tile([128, 16, 64], f32, tag=f"ot{s2}",
                                 name=f"ot_{p}_{s2}") for s2 in range(2)]
                accA = accp.tile([128, 7, 65], f32, tag="accA", name=f"accA_{p}")
                accB = accp.tile([128, 7, 65], f32, tag="accB", name=f"accB_{p}")
                accC = accp.tile([128, 2, 65], f32, tag="accC", name=f"accC_{p}")
                accS = accp.tile([128, 7, 65], f32, tag="accS", name=f"accS_{p}")

                def accs_for(s2, qs, _t=(accA, accB, accC, accS)):
                    aA, aB, aC, aS = _t
                    if s2 == 0:
                        if qs < 7:
                            return aA[:, qs, :]
                        if qs < 14:
                            return aB[:, qs - 7, :]
                        return aC[:, qs - 14, :]
                    assert 9 <= qs < 16
                    return aS[:, qs - 9, :]

                def emit_fill_v3(kb, c):
                    w = S - 128 * kb
                    q0 = 128 * kb
                    cw = min(512, w - c)
                    st = stp.tile([128, 1024], f32, tag="st",
                                  name=f"st_{p}_{kb}_{c}")
                    for s2 in range(2):
                        po = 64 * s2
                        nc.tensor.matmul(
                            st[:, 512 * s2:512 * s2 + cw],
                            lhsT=kt_t[po:po + 64, q0:q0 + 128],
                            rhs=qt_t[po:po + 64, q0 + c:q0 + c + cw],
                            start=True, stop=True,
                        )
                    if MODE == "qk_only":
                        return
                    src = st.rearrange("r (s q) -> r s q", s=2)[:, :, 0:cw]
                    dst = pts[:, :, offs[kb] + c:offs[kb] + c + cw]
                    nc.scalar.activation(dst, src, Exp, scale=0.125)
                    if c == 0:
                        d = pts[:, :, offs[kb]:offs[kb] + 128]
                        nc.vector.tensor_mul(d, d, tri2[:, :, :])

                def emit_fill_v4(fi):
                    """Packed fill fi: stream cols [512*fi, 512*(fi+1)) of the
                    concatenated causal strips; may span strip boundaries."""
                    lo, hi = 512 * fi, 512 * (fi + 1)
                    st = stp.tile([128, 1024], f32, tag="st",
                                  name=f"st_{p}_{fi}")
                    segs = []
                    for kb in range(KB):
                        a = max(lo, offs[kb])
                        b = min(hi, offs[kb] + (S - 128 * kb))
                        if a < b:
                            segs.append((kb, a, b))
                    for kb, a, b in segs:
                        q0 = 128 * kb
                        for s2 in range(2):
                            po = 64 * s2
                            nc.tensor.matmul(
                                st[:, 512 * s2 + a - lo:512 * s2 + b - lo],
                                lhsT=kt_t[po:po + 64, q0:q0 + 128],
                                rhs=qt_t[po:po + 64,
                                         q0 + a - offs[kb]:q0 + b - offs[kb]],
                                start=True, stop=True,
                            )
                    if MODE == "qk_only":
                        return
                    src = st.rearrange("r (s q) -> r s q", s=2)[:, :, :]
                    nc.scalar.activation(pts[:, :, lo:hi], src, Exp, scale=0.125)
                    for kb, a, b in segs:
                        # tri-mask the diag block once its last column is exp'd
                        dend = offs[kb] + 128
                        if a <= dend - 1 < b:
                            d = pts[:, :, offs[kb]:dend]
                            nc.vector.tensor_mul(d, d, tri2[:, :, :])

                def pv(s2, kb, qs, acc_ap, start, stop):
                    h = 2 * p + s2
                    m = offs[kb] + 128 * (qs - kb)
                    nc.tensor.matmul(
                        acc_ap,
                        lhsT=pts[:, s2, m:m + 128],
                        rhs=vg_t[:, h, kb, :],
                        start=start, stop=stop,
                    )

                def emit_burst(kb):
                    # head A: all open chains; diag qs==kb last (tri dep)
                    qs_range = list(range(kb, 16))
                    if kb > 0:
                        qs_range = qs_range[1:] + [kb]
                    for qs in qs_range:
                        pv(0, kb, qs, accs_for(0, qs),
                           start=(kb == 0 and qs in (0, 7, 14)),
                           stop=(qs in (6, 13, 15) and kb == qs))
                    # head B eager subset: qs 9..15 in the spare bank
                    bq = [qs for qs in range(max(kb, 9), 16)]
                    if kb > 9 and bq and bq[0] == kb:
                        bq = bq[1:] + [kb]
                    for qs in bq:
                        pv(1, kb, qs, accs_for(1, qs),
                           start=(kb == 0 and qs == 9),
                           stop=(qs == 15 and kb == qs))

                def norm(s2, acc_t, col0, qs_lo, n):
                    ot = ots[s2]
                    rs = small.tile([128, n], f32, tag="rs",
                                    name=f"rs_{p}_{s2}_{qs_lo}")
                    nc.vector.reciprocal(rs[:, :], acc_t[:, col0:col0 + n, 64])
                    for j in range(n):
                        nc.vector.tensor_scalar_mul(
                            ot[:, qs_lo + j, :], acc_t[:, col0 + j, 0:64],
                            rs[:, j:j + 1])

                def dma_rows(s2, qs_lo, qs_hi):
                    h = 2 * p + s2
                    dst = o[h, qs_lo * 128:qs_hi * 128, :].rearrange(
                        "(j r) c -> r j c", r=128)
                    nc.sync.dma_start(dst, ots[s2][:, qs_lo:qs_hi, :])

                full = MODE == "full"
                cell = {}

                def post_burst(j):
                    # bank-group completions: normalize + store the moment a
                    # group's last chain closes, so nothing piles up at the end
                    if j == 6:
                        norm(0, accA, 0, 0, 7)
                        dma_rows(0, 0, 7)
                        # B's qs0-6 chains into A's freed bank
                        accA2 = accp.tile([128, 7, 65], f32, tag="accA",
                                          name=f"accA2_{p}")
                        for kb2 in range(0, 7):
                            for qs in range(kb2, 7):
                                pv(1, kb2, qs, accA2[:, qs, :],
                                   start=(kb2 == 0 and qs == 0),
                                   stop=(qs == 6 and kb2 == qs))
                        norm(1, accA2, 0, 0, 7)
                        dma_rows(1, 0, 7)
                    if j == 13:
                        norm(0, accB, 0, 7, 7)
                        dma_rows(0, 7, 14)
                    if j == 14:
                        # B's accS chains qs9-14 are closed (cols 0..5)
                        norm(1, accS, 0, 9, 6)

                if VERSION >= 4:
                    end_fill = {}
                    for kb in range(KB):
                        e = (offs[kb + 1] - 1 if kb + 1 < KB else W - 1) // 512
                        end_fill.setdefault(e, []).append(kb)
                    for fi in range(W // 512):
                        emit_fill_v4(fi)
                        if not full:
                            continue
                        for kb in end_fill.get(fi, []):
                            if kb >= 1:
                                emit_burst(kb - 1)
                                post_burst(kb - 1)
                else:
                    for kb in range(KB):
                        for c in range(0, S - 128 * kb, 512):
                            emit_fill_v3(kb, c)
                        if full and kb >= 1:
                            emit_burst(kb - 1)
                            post_burst(kb - 1)
                if full:
                    emit_burst(KB - 1)
                    norm(0, accC, 0, 14, 2)
                    dma_rows(0, 14, 16)
                    norm(1, accS, 6, 15, 1)
                    dma_rows(1, 9, 16)
                    # B's qs7-8 chains into A's freed accC bank
                    accC2 = accp.tile([128, 2, 65], f32, tag="accC",
                                      name=f"accC2_{p}")
                    for kb2 in range(0, 9):
                        for qs in (7, 8):
                            if qs < kb2:
                                continue
                            pv(1, kb2, qs, accC2[:, qs - 7, :],
                               start=(kb2 == 0 and qs == 7),
                               stop=(qs == 8 and kb2 == qs))
                    norm(1, accC2, 0, 7, 2)
                    dma_rows(1, 7, 9)


def _emit(tc, nc, mybir, qt, kt, vg, tri, o, causal, reps=1):
    from contextlib import ExitStack

    f32 = mybir.dt.float32
    bf = mybir.dt.bfloat16
    Exp = mybir.ActivationFunctionType.Exp

    with ExitStack() as ctx:
        const = ctx.enter_context(tc.tile_pool(name="const", bufs=1))
        qk = ctx.enter_context(tc.tile_pool(name="qk", bufs=2))
        vpool = ctx.enter_context(tc.tile_pool(name="vp", bufs=2))
        ptp = ctx.enter_context(tc.tile_pool(name="ptp", bufs=PT_BUFS))
        outp = ctx.enter_context(tc.tile_pool(name="outp", bufs=4))
        small = ctx.enter_context(tc.tile_pool(name="small", bufs=4))
        stp = ctx.enter_context(tc.tile_pool(name="stp", bufs=ST_BUFS, space="PSUM"))
        accp = ctx.enter_context(tc.tile_pool(name="accp", bufs=1, space="PSUM"))

        tri_t = const.tile([128, 128], bf, name="tri_t")
        nc.sync.dma_start(tri_t[:, :], tri[:, :])

        # dummy exp issued first: walrus places the ~2.7us ACT table load
        # before the first ACTIVATE in the stream, so doing one on a tiny
        # constant tile overlaps the table load with the input DMAs instead
        # of serializing it before the first real exp
        warm = const.tile([128, 1], f32, name="warm")
        nc.vector.memset(warm[:, :], 0.0)
        nc.scalar.activation(warm[:, :], warm[:, :], Exp)

        from contextlib import nullcontext
        with (tc.For_i(0, reps, 1, hint_engines=(mybir.EngineType.PE,),
                       staggered_reset=STAGGERED)
              if reps > 1 else nullcontext()):
          rep = 0  # body emitted once; hardware loop repeats it
          # All inputs for the whole iteration are loaded up front, spread
          # across three DMA rings (SP + ACT HWDGE, gpsimd SWDGE) so nothing
          # downstream ever waits on a load except the very first strip:
          #   SP:  kt head-block for pair0 (tiny, unblocks QK(0) fast), qt
          #   ACT: kt bulk
          #   SWDGE: vg for all four heads (one batched start)
          qt_ts, kt_ts = [], []
          vg_t = vpool.tile([128, HEADS_PER_CORE, KB, 65], bf, tag="vg",
                            name=f"vg_{rep}")
          for p in range(HEADS_PER_CORE // 2):
              qt_t = qk.tile([128, S], bf, tag=f"qt{p}", name=f"qt_{rep}_{p}")
              kt_t = qk.tile([128, S], bf, tag=f"kt{p}", name=f"kt_{rep}_{p}")
              nc.sync.dma_start(kt_t[:, :128], kt[p][:, :128])
              nc.sync.dma_start(qt_t[:, :], qt[p][:, :])
              nc.scalar.dma_start(kt_t[:, 128:], kt[p][:, 128:])
              qt_ts.append(qt_t)
              kt_ts.append(kt_t)
          vg_src = vg.rearrange("h p k c -> p h k c")
          nc.gpsimd.dma_start(vg_t[:, 0:1, :, :], vg_src[:, 0:1, :, :])
          nc.gpsimd.dma_start(vg_t[:, 1:, :, :], vg_src[:, 1:, :, :])
          for p in range(HEADS_PER_CORE // 2):
              qt_t = qt_ts[p]
              kt_t = kt_ts[p]
              for s2 in range(2):
                  h = 2 * p + s2
                  po = 64 * s2  # partition offset of this head's d-dim

                  accA = accp.tile([128, 7, 65], f32, tag="accA", name=f"accA_{rep}_{h}")
                  accB = accp.tile([128, 7, 65], f32, tag="accB", name=f"accB_{rep}_{h}")
                  accC = accp.tile([128, 2, 65], f32, tag="accC", name=f"accC_{rep}_{h}")

                  def acc(i):
                      if i < 7:
                          return accA[:, i, :]
                      if i < 14:
                          return accB[:, i - 7, :]
                      return accC[:, i - 14, :]

                  def strip_halves(kb):
                      q0 = 128 * kb if causal else 0
                      cols = S - q0
                      pieces = []
                      hs = 0
                      while hs < cols:
                          pieces.append((q0, hs, min(1024, cols - hs), "A"))
                          hs += 1024
                      return pieces

                  def emit_qk(kb):
                      """QK matmuls for strip kb; returns the st tiles."""
                      sts = []
                      for q0, hs, hw, tg in strip_halves(kb):
                          st = stp.tile([128, 1024], f32, tag="st",
                                        name=f"st_{rep}_{h}_{kb}_{hs}")
                          for c0 in range(0, hw, 512):
                              cw = min(512, hw - c0)
                              nc.tensor.matmul(
                                  st[:, c0:c0 + cw],
                                  lhsT=kt_t[po:po + 64, kb * 128:(kb + 1) * 128],
                                  rhs=qt_t[po:po + 64,
                                           q0 + hs + c0:q0 + hs + c0 + cw],
                                  start=True, stop=True,
                              )
                          sts.append(st)
                      return sts

                  def normalize_bank(qs_lo, qs_hi):
                      """Normalize q_subs [qs_lo, qs_hi) and write out in one
                      batched DMA (rows r of out tile j map to q = qs*128+r)."""
                      n = qs_hi - qs_lo
                      ot = outp.tile([128, n, 64], f32, tag=f"ot{qs_lo}",
                                     name=f"ot_{rep}_{h}_{qs_lo}")
                      for j in range(n):
                          a = acc(qs_lo + j)
                          rs = small.tile([128, 1], f32, tag="rs",
                                          name=f"rs_{rep}_{h}_{qs_lo + j}")
                          nc.vector.reciprocal(rs[:, :], a[:, 64:65])
                          nc.vector.tensor_scalar_mul(ot[:, j, :], a[:, :64],
                                                      rs[:, :])
                      dst = o[h, qs_lo * 128:qs_hi * 128, :].rearrange(
                          "(j r) c -> r j c", r=128)
                      nc.sync.dma_start(dst, ot[:, :, :])

                  sts = emit_qk(0)
                  for kb in range(KB):
                      # exp of strip kb
                      pts = []
                      for (q0, hs, hw, tg), st in zip(strip_halves(kb), sts):
                          if MODE == "qk_only":
                              continue
                          pt = ptp.tile([128, 1024], bf, tag="pt",
                                        name=f"pt_{rep}_{h}_{kb}_{hs}")
                          nc.scalar.activation(pt[:, :hw], st[:, :hw], Exp,
                                               scale=0.125)
                          if causal and hs == 0:
                              nc.vector.tensor_mul(pt[:, :128], pt[:, :128],
                                                   tri_t[:, :])
                          pts.append(pt)
                      # QK for strip kb+1 goes to PE before PV of strip kb so
                      # the PE never stalls behind ACT
                      if kb + 1 < KB:
                          sts = emit_qk(kb + 1)
                      # PV accumulation for strip kb
                      if MODE != "full":
                          continue
                      for (q0, hs, hw, tg), pt in zip(strip_halves(kb), pts):
                          qs_range = list(range((q0 + hs) // 128,
                                                (q0 + hs + hw) // 128))
                          # the diagonal q_sub (== kb) additionally depends on
                          # the DVE tri-multiply; emit it last so the PE can
                          # start the other PV matmuls as soon as exp is done.
                          # (at kb==0 keep ascending order: the bank-group
                          # start=True matmuls must be first into each bank)
                          if causal and kb > 0 and qs_range and qs_range[0] == kb:
                              qs_range = qs_range[1:] + [kb]
                          for q_sub in qs_range:
                              m = q_sub * 128 - q0 - hs
                              last_kb = q_sub if causal else KB - 1
                              nc.tensor.matmul(
                                  acc(q_sub),
                                  lhsT=pt[:, m:m + 128],
                                  rhs=vg_t[:, h, kb, :],
                                  start=(kb == 0 and q_sub in _BANK_FIRST),
                                  stop=(q_sub in _BANK_LAST and kb == last_kb),
                              )
                      # normalize accumulator banks as soon as they complete
                      if causal:
                          if kb == 6:
                              normalize_bank(0, 7)
                          elif kb == 13:
                              normalize_bank(7, 14)
                          elif kb == 15:
                              normalize_bank(14, 16)
                  if not causal:
                      normalize_bank(0, 7)
                      normalize_bank(7, 14)
                      normalize_bank(14, 16)


def build_nc(causal=True, reps=1):
    """Build + compile the per-core Bass program (cached)."""
    key = ("nc", causal, reps, STRIP_W, MODE, ST_BUFS, PT_BUFS, STAGGERED,
           VERSION)
    if key in _built:
        return _built[key]
    import concourse.bacc as bacc
    from concourse import mybir, tile

    nc = bacc.Bacc("TRN2", target_bir_lowering=False, debug=False,
                   num_devices=N_CORES)
    qt = nc.dram_tensor("qt", (HEADS_PER_CORE // 2, 128, S),
                        mybir.dt.bfloat16, kind="ExternalInput").ap()
    kt = nc.dram_tensor("kt", (HEADS_PER_CORE // 2, 128, S),
                        mybir.dt.bfloat16, kind="ExternalInput").ap()
    vg = nc.dram_tensor("vg", (HEADS_PER_CORE, 128, KB, 65),
                        mybir.dt.bfloat16, kind="ExternalInput").ap()
    tri = nc.dram_tensor("tri", (128, 128), mybir.dt.bfloat16,
                         kind="ExternalInput").ap()
    o = nc.dram_tensor("o", (HEADS_PER_CORE, S, D), mybir.dt.float32,
                       kind="ExternalOutput").ap()
    with tile.TileContext(nc) as tc:
        if VERSION >= 3 and causal:
            _emit_v3(tc, nc, mybir, qt, kt, vg, tri, o, reps)
        else:
            _emit(tc, nc, mybir, qt, kt, vg, tri, o, causal, reps)
    nc.compile()
    _built[key] = nc
    return nc


def prep_inputs(Q, K, V):
    """Host-side shard + layout prep. Returns list of 8 per-core input dicts."""
    Qf = np.ascontiguousarray(Q, dtype=np.float32).reshape(B * H, S, D)
    Kf = np.ascontiguousarray(K, dtype=np.float32).reshape(B * H, S, D)
    Vf = np.ascontiguousarray(V, dtype=np.float32).reshape(B * H, S, D)

    # [BH, S, D] -> transposed, bf16: [BH, D, S]
    Qt = np.ascontiguousarray(Qf.transpose(0, 2, 1)).astype(_BF16)
    Kt = np.ascontiguousarray(Kf.transpose(0, 2, 1)).astype(_BF16)

    # V augmented with ones column, partition-major: [BH, 128, KB, 65]
    Vb = Vf.astype(_BF16)
    vg_all = np.empty((B * H, 128, KB, 65), dtype=_BF16)
    # V[h, kb*128 + r, c] -> vg[h, r, kb, c]
    vg_all[:, :, :, :64] = Vb.reshape(B * H, KB, 128, D).transpose(0, 2, 1, 3)
    vg_all[:, :, :, 64] = _BF16(1.0)

    tri_np = (np.tril(np.ones((128, 128), dtype=np.float32))
              .T.astype(_BF16))  # tri[k, c] = 1 if c >= k
    tri_np = np.ascontiguousarray(tri_np)

    in_maps = []
    for c in range(N_CORES):
        h0 = c * HEADS_PER_CORE
        qt_c = np.empty((HEADS_PER_CORE // 2, 128, S), dtype=_BF16)
        kt_c = np.empty((HEADS_PER_CORE // 2, 128, S), dtype=_BF16)
        for p in range(HEADS_PER_CORE // 2):
            qt_c[p, :64] = Qt[h0 + 2 * p]
            qt_c[p, 64:] = Qt[h0 + 2 * p + 1]
            kt_c[p, :64] = Kt[h0 + 2 * p]
            kt_c[p, 64:] = Kt[h0 + 2 * p + 1]
        in_maps.append({
            "qt": qt_c,
            "kt": kt_c,
            "vg": np.ascontiguousarray(vg_all[h0:h0 + HEADS_PER_CORE]),
            "tri": tri_np,
        })
    return in_maps


def _classify_mask(mask):
    m = np.asarray(mask).reshape(S, S)
    if not m.any():
        return "dense"
    if np.array_equal(m, np.triu(np.ones((S, S), dtype=bool), k=1)):
        return "causal"
    raise NotImplementedError("only causal or all-False masks supported")


def run_cores(in_maps, causal=True, reps=1, **kwargs):
    from concourse import bass_utils

    nc = build_nc(causal, reps)
    return bass_utils.run_bass_kernel_spmd(
        nc, in_maps, core_ids=list(range(N_CORES)), **kwargs
    )


def kernel(Q, K, V, mask):
    kind = _classify_mask(mask)
    in_maps = prep_inputs(Q, K, V)
    res = run_cores(in_maps, causal=(kind == "causal"))
    out = np.concatenate([r["o"] for r in res.results], axis=0)
    return out.reshape(B, H, S, D).astype(np.float32)


if __name__ == "__main__":
    rng = np.random.default_rng(0)
    Q = rng.standard_normal((B, H, S, D), dtype=np.float32)
    K = rng.standard_normal((B, H, S, D), dtype=np.float32)
    V = rng.standard_normal((B, H, S, D), dtype=np.float32)
    mask = np.triu(np.ones((S, S), dtype=bool), k=1)[None, None]
    out = kernel(Q, K, V, mask)
    print("out", out.shape, out.dtype)



# revision 34
# speedup vs baseline: 1.0067x; 1.0067x over previous
"""Causal multi-head attention kernel for Trainium2 (8 NeuronCores).

Problem: B=2, H=16, S=2048, D=64 causal attention (softmax over last axis).
Sharding: 32 (batch, head) pairs split 4-per-core across 8 cores; each core
computes its heads independently (no collectives).

v3 per-core algorithm (pair-interleaved, S-transposed layout; see _emit_v3):
  - Host pre-packs qt/kt with the two heads of a pair on SBUF partitions
    0-63 / 64-127.  Their K=64 QK matmuls auto-derive PE tile_position
    (0,0)/(64,0) from base partitions, so adjacent A/B matmuls run
    CONCURRENTLY on the PE row-groups (measured exactly 2x).
  - Each 512-col fill writes [A|B] halves of one [128,1024] f32 PSUM strip
    slot (2 slots, 4 banks), and ONE joint ACTIVATE exps BOTH heads
    (FD=1024) into a per-pair P^T store in SBUF.  ACT per-instruction
    overhead is ~430 ns on HW, so instruction count dominates the exp cost;
    the joint exps cut it from 96 to 80 per core.
  - PV kb-outer: head A's 16 chains eager in 3 PSUM banks (7/7/2 + ones-col
    row-sum trick); head B's qs9-15 chains eager in the spare 8th bank;
    B's qs0-6 / qs7-8 chains reuse A's banks as A's bank-group
    normalizations release them.  Every bank group is normalized (batched
    DVE reciprocal + per-row scalar mul) and DMA'd out the moment it closes.
  - Input DMAs are batched per iteration and spread across the SP HWDGE,
    ACT HWDGE, and gpsimd SWDGE rings.

Measured per-iteration HW time (8-core SPMD, repeat-differenced): ~94 us
vs 127-134 us for the per-head sequential baseline (VERSION=2 path).

kernel(Q, K, V, mask) takes the full unsharded fp32 inputs and returns the
full [2, 16, 2048, 64] fp32 output.
"""

import sys

if "/opt/trn_rl_repo" not in sys.path:
    sys.path.insert(0, "/opt/trn_rl_repo")

import numpy as np
import ml_dtypes

B, H, S, D = 2, 16, 2048, 64
N_CORES = 8
HEADS_PER_CORE = (B * H) // N_CORES  # 4
KB = S // 128  # 16 k-blocks per head
QS = S // 128  # 16 q-subblocks per head

_BF16 = ml_dtypes.bfloat16

# accumulator bank packing: q_subs 0-6 -> bank A, 7-13 -> bank B, 14-15 -> C
_BANK_FIRST = (0, 7, 14)   # first q_sub written in each accumulator bank
_BANK_LAST = (6, 13, 15)   # last q_sub written in each accumulator bank

_built = {}
STRIP_W = 1024
MODE = "full"  # full | qk_only | qk_exp (timing ablations)
ST_BUFS = 2  # PSUM strip-tile slots  # S^T strip tile width (PSUM free elems)
PT_BUFS = 4   # SBUF P^T tile slots (exp outputs)
STAGGERED = True  # staggered_reset on the timing loop (overlaps iterations)
VERSION = 3   # 2 = per-head sequential; 3 = pair-interleaved row-tiled (BEST)
              # 4 = v3 + packed fills (SLOWER: PE LDW thrash > ACT savings)
              # 5 = v3 + 3-bank accs -> 5 strip banks, 1536/1024 slots
              #     (FAULTS on HW: A/B row-tiled QK matmuls of a 1536-wide
              #      slot land concurrently in the same PSUM bank)


def _emit_v3(tc, nc, mybir, qt, kt, vg, tri, o, reps=1):
    """Pair-interleaved causal attention.

    The two heads of a pair occupy SBUF partitions 0-63 (A) and 64-127 (B) of
    qt/kt.  Their QK matmuls carry tile_position (0,0)/(64,0) automatically
    (base partitions), so adjacent A/B matmuls run CONCURRENTLY on the PE's
    row-groups.  Each 512-col fill writes [A|B] side by side into one
    [128,1024] PSUM strip slot, and ONE joint ACTIVATE exps both heads into a
    per-pair P^T store in SBUF (pts).  PV for head A runs kb-outer eagerly
    (accs in 3 banks); head B's qs7-13 chains run eagerly in the spare 8th
    bank, and its qs0-6 / qs14-15 chains reuse A's banks as A's
    normalizations release them.  PSUM: 2x2-bank strip slots + 4 acc banks =
    exactly 8.
    """
    from contextlib import ExitStack, nullcontext

    f32 = mybir.dt.float32
    bf = mybir.dt.bfloat16
    Exp = mybir.ActivationFunctionType.Exp

    offs = []
    off = 0
    for kb in range(KB):
        offs.append(off)
        off += S - 128 * kb
    W = off  # 17408

    with ExitStack() as ctx:
        const = ctx.enter_context(tc.tile_pool(name="const", bufs=1))
        qk = ctx.enter_context(tc.tile_pool(name="qk", bufs=1))
        vpool = ctx.enter_context(tc.tile_pool(name="vp", bufs=1))
        ptsp = ctx.enter_context(tc.tile_pool(name="ptsp", bufs=2))
        outp = ctx.enter_context(tc.tile_pool(name="outp", bufs=2))
        small = ctx.enter_context(tc.tile_pool(name="small", bufs=4))
        stp = ctx.enter_context(tc.tile_pool(name="stp", bufs=ST_BUFS, space="PSUM"))
        accp = ctx.enter_context(tc.tile_pool(name="accp", bufs=1, space="PSUM"))

        tri2 = const.tile([128, 2, 128], bf, name="tri2")
        nc.gpsimd.dma_start(tri2[:, 0, :], tri[:, :])
        nc.gpsimd.dma_start(tri2[:, 1, :], tri[:, :])

        # dummy exp: hoists the ~2.7us ACT table load ahead of the input DMAs
        warm = const.tile([128, 1], f32, name="warm")
        nc.vector.memset(warm[:, :], 0.0)
        nc.scalar.activation(warm[:, :], warm[:, :], Exp)

        with (tc.For_i(0, reps, 1, hint_engines=(mybir.EngineType.PE,),
                       staggered_reset=STAGGERED)
              if reps > 1 else nullcontext()):
            # iteration-wide input loads, spread over SP/ACT/SWDGE rings
            qt_ts, kt_ts = [], []
            vg_t = vpool.tile([128, HEADS_PER_CORE, KB, 65], bf, tag="vg",
                              name="vg")
            for p in range(HEADS_PER_CORE // 2):
                qt_t = qk.tile([128, S], bf, tag=f"qt{p}", name=f"qt_{p}")
                kt_t = qk.tile([128, S], bf, tag=f"kt{p}", name=f"kt_{p}")
                nc.sync.dma_start(kt_t[:, :128], kt[p][:, :128])
                if p == 0:
                    nc.sync.dma_start(qt_t[:, :512], qt[p][:, :512])
                    nc.sync.dma_start(qt_t[:, 512:], qt[p][:, 512:])
                else:
                    nc.sync.dma_start(qt_t[:, :], qt[p][:, :])
                nc.scalar.dma_start(kt_t[:, 128:], kt[p][:, 128:])
                qt_ts.append(qt_t)
                kt_ts.append(kt_t)
            vg_src = vg.rearrange("h p k c -> p h k c")
            nc.gpsimd.dma_start(vg_t[:, 0:1, :, :], vg_src[:, 0:1, :, :])
            nc.gpsimd.dma_start(vg_t[:, 1:, :, :], vg_src[:, 1:, :, :])

            for p in range(HEADS_PER_CORE // 2):
                qt_t, kt_t = qt_ts[p], kt_ts[p]
                pts = ptsp.tile([128, 2, W], bf, tag="pts", name=f"pts_{p}")
                ots = [outp.tile([128, 16, 64], f32, tag=f"ot{s2}",
                                 name=f"ot_{p}_{s2}") for s2 in range(2)]
                accA = accp.tile([128, 7, 65], f32, tag="accA", name=f"accA_{p}")
                accB = accp.tile([128, 7, 65], f32, tag="accB", name=f"accB_{p}")
                accC = accp.tile([128, 2, 65], f32, tag="accC", name=f"accC_{p}")
                accS = accp.tile([128, 7, 65], f32, tag="accS", name=f"accS_{p}")

                def accs_for(s2, qs, _t=(accA, accB, accC, accS)):
                    aA, aB, aC, aS = _t
                    if s2 == 0:
                        if qs < 7:
                            return aA[:, qs, :]
                        if qs < 14:
                            return aB[:, qs - 7, :]
                        return aC[:, qs - 14, :]
                    assert 9 <= qs < 16
                    return aS[:, qs - 9, :]

                def emit_fill_v3(kb, c):
                    w = S - 128 * kb
                    q0 = 128 * kb
                    cw = min(512, w - c)
                    st = stp.tile([128, 1024], f32, tag="st",
                                  name=f"st_{p}_{kb}_{c}")
                    for s2 in range(2):
                        po = 64 * s2
                        nc.tensor.matmul(
                            st[:, 512 * s2:512 * s2 + cw],
                            lhsT=kt_t[po:po + 64, q0:q0 + 128],
                            rhs=qt_t[po:po + 64, q0 + c:q0 + c + cw],
                            start=True, stop=True,
                        )
                    if MODE == "qk_only":
                        return
                    src = st.rearrange("r (s q) -> r s q", s=2)[:, :, 0:cw]
                    dst = pts[:, :, offs[kb] + c:offs[kb] + c + cw]
                    nc.scalar.activation(dst, src, Exp, scale=0.125)
                    if c == 0:
                        d = pts[:, :, offs[kb]:offs[kb] + 128]
                        nc.vector.tensor_mul(d, d, tri2[:, :, :])

                def emit_fill_v4(fi):
                    """Packed fill fi: stream cols [512*fi, 512*(fi+1)) of the
                    concatenated causal strips; may span strip boundaries."""
                    lo, hi = 512 * fi, 512 * (fi + 1)
                    st = stp.tile([128, 1024], f32, tag="st",
                                  name=f"st_{p}_{fi}")
                    segs = []
                    for kb in range(KB):
                        a = max(lo, offs[kb])
                        b = min(hi, offs[kb] + (S - 128 * kb))
                        if a < b:
                            segs.append((kb, a, b))
                    for kb, a, b in segs:
                        q0 = 128 * kb
                        for s2 in range(2):
                            po = 64 * s2
                            nc.tensor.matmul(
                                st[:, 512 * s2 + a - lo:512 * s2 + b - lo],
                                lhsT=kt_t[po:po + 64, q0:q0 + 128],
                                rhs=qt_t[po:po + 64,
                                         q0 + a - offs[kb]:q0 + b - offs[kb]],
                                start=True, stop=True,
                            )
                    if MODE == "qk_only":
                        return
                    src = st.rearrange("r (s q) -> r s q", s=2)[:, :, :]
                    nc.scalar.activation(pts[:, :, lo:hi], src, Exp, scale=0.125)
                    for kb, a, b in segs:
                        # tri-mask the diag block once its last column is exp'd
                        dend = offs[kb] + 128
                        if a <= dend - 1 < b:
                            d = pts[:, :, offs[kb]:dend]
                            nc.vector.tensor_mul(d, d, tri2[:, :, :])

                def pv(s2, kb, qs, acc_ap, start, stop):
                    h = 2 * p + s2
                    m = offs[kb] + 128 * (qs - kb)
                    nc.tensor.matmul(
                        acc_ap,
                        lhsT=pts[:, s2, m:m + 128],
                        rhs=vg_t[:, h, kb, :],
                        start=start, stop=stop,
                    )

                def emit_burst(kb):
                    # head A: all open chains; diag qs==kb last (tri dep)
                    qs_range = list(range(kb, 16))
                    if kb > 0:
                        qs_range = qs_range[1:] + [kb]
                    for qs in qs_range:
                        pv(0, kb, qs, accs_for(0, qs),
                           start=(kb == 0 and qs in (0, 7, 14)),
                           stop=(qs in (6, 13, 15) and kb == qs))
                    # head B eager subset: qs 9..15 in the spare bank
                    bq = [qs for qs in range(max(kb, 9), 16)]
                    if kb > 9 and bq and bq[0] == kb:
                        bq = bq[1:] + [kb]
                    for qs in bq:
                        pv(1, kb, qs, accs_for(1, qs),
                           start=(kb == 0 and qs == 9),
                           stop=(qs == 15 and kb == qs))

                def norm(s2, acc_t, col0, qs_lo, n):
                    ot = ots[s2]
                    rs = small.tile([128, n], f32, tag="rs",
                                    name=f"rs_{p}_{s2}_{qs_lo}")
                    nc.vector.reciprocal(rs[:, :], acc_t[:, col0:col0 + n, 64])
                    for j in range(n):
                        nc.vector.tensor_scalar_mul(
                            ot[:, qs_lo + j, :], acc_t[:, col0 + j, 0:64],
                            rs[:, j:j + 1])

                def dma_rows(s2, qs_lo, qs_hi):
                    h = 2 * p + s2
                    dst = o[h, qs_lo * 128:qs_hi * 128, :].rearrange(
                        "(j r) c -> r j c", r=128)
                    nc.sync.dma_start(dst, ots[s2][:, qs_lo:qs_hi, :])

                full = MODE == "full"

                def post_burst(j):
                    # bank-group completions: normalize + store the moment a
                    # group's last chain closes, so nothing piles up at the end
                    if j == 6:
                        norm(0, accA, 0, 0, 7)
                        dma_rows(0, 0, 7)
                        # B's qs0-6 chains into A's freed bank
                        accA2 = accp.tile([128, 7, 65], f32, tag="accA",
                                          name=f"accA2_{p}")
                        for kb2 in range(0, 7):
                            for qs in range(kb2, 7):
                                pv(1, kb2, qs, accA2[:, qs, :],
                                   start=(kb2 == 0 and qs == 0),
                                   stop=(qs == 6 and kb2 == qs))
                        norm(1, accA2, 0, 0, 7)
                        dma_rows(1, 0, 7)
                    if j == 13:
                        norm(0, accB, 0, 7, 7)
                        dma_rows(0, 7, 14)
                    if j == 14:
                        # B's accS chains qs9-14 are closed (cols 0..5)
                        norm(1, accS, 0, 9, 6)

                if VERSION >= 4:
                    end_fill = {}
                    for kb in range(KB):
                        e = (offs[kb + 1] - 1 if kb + 1 < KB else W - 1) // 512
                        end_fill.setdefault(e, []).append(kb)
                    for fi in range(W // 512):
                        emit_fill_v4(fi)
                        if not full:
                            continue
                        for kb in end_fill.get(fi, []):
                            if kb >= 1:
                                emit_burst(kb - 1)
                                post_burst(kb - 1)
                else:
                    for kb in range(KB):
                        for c in range(0, S - 128 * kb, 512):
                            emit_fill_v3(kb, c)
                        if full and kb >= 1:
                            emit_burst(kb - 1)
                            post_burst(kb - 1)
                if full:
                    emit_burst(KB - 1)
                    norm(0, accC, 0, 14, 2)
                    dma_rows(0, 14, 16)
                    norm(1, accS, 6, 15, 1)
                    dma_rows(1, 9, 16)
                    # B's qs7-8 chains into A's freed accC bank
                    accC2 = accp.tile([128, 2, 65], f32, tag="accC",
                                      name=f"accC2_{p}")
                    for kb2 in range(0, 9):
                        for qs in (7, 8):
                            if qs < kb2:
                                continue
                            pv(1, kb2, qs, accC2[:, qs - 7, :],
                               start=(kb2 == 0 and qs == 7),
                               stop=(qs == 8 and kb2 == qs))
                    norm(1, accC2, 0, 7, 2)
                    dma_rows(1, 7, 9)


def _emit_v5(tc, nc, mybir, qt, kt, vg, tri, o, reps=1):
    """v3 + 3-bank accumulator choreography freeing a 5th strip bank.

    Strip slots become an asymmetric ping-pong pair [128,1536]+[128,1024]
    ([A768|B768] / [A512|B512] fills), cutting joint-exp ACTIVATEs from 40
    to ~30 per head-pair (ACT per-instruction overhead is ~430 ns, so this
    is ~4.3 us/core of ACT busy).  Accumulator banks:
      bank accA: t1 = A qs0-6 eager; t2 (after kb6) = A qs14-15 + B qs0-4
      bank accB: t1 = A qs7-13 eager; t2 (after kb13) = B qs5-8
      bank accS: B qs9-15 eager
    A qs14-15 prefix chains (kb0-6) catch up in one block at kb6, then ride
    the regular bursts; single-start-per-bank ordering is preserved.
    """
    from contextlib import ExitStack, nullcontext

    f32 = mybir.dt.float32
    bf = mybir.dt.bfloat16
    Exp = mybir.ActivationFunctionType.Exp

    offs = []
    off = 0
    for kb in range(KB):
        offs.append(off)
        off += S - 128 * kb
    W = off  # 17408

    with ExitStack() as ctx:
        const = ctx.enter_context(tc.tile_pool(name="const", bufs=1))
        qk = ctx.enter_context(tc.tile_pool(name="qk", bufs=1))
        vpool = ctx.enter_context(tc.tile_pool(name="vp", bufs=1))
        ptsp = ctx.enter_context(tc.tile_pool(name="ptsp", bufs=2))
        outp = ctx.enter_context(tc.tile_pool(name="outp", bufs=2))
        small = ctx.enter_context(tc.tile_pool(name="small", bufs=4))
        stp = ctx.enter_context(tc.tile_pool(name="stp", bufs=1, space="PSUM"))
        accp = ctx.enter_context(tc.tile_pool(name="accp", bufs=1, space="PSUM"))

        tri2 = const.tile([128, 2, 128], bf, name="tri2")
        nc.gpsimd.dma_start(tri2[:, 0, :], tri[:, :])
        nc.gpsimd.dma_start(tri2[:, 1, :], tri[:, :])

        warm = const.tile([128, 1], f32, name="warm")
        nc.vector.memset(warm[:, :], 0.0)
        nc.scalar.activation(warm[:, :], warm[:, :], Exp)

        with (tc.For_i(0, reps, 1, hint_engines=(mybir.EngineType.PE,),
                       staggered_reset=STAGGERED)
              if reps > 1 else nullcontext()):
            qt_ts, kt_ts = [], []
            vg_t = vpool.tile([128, HEADS_PER_CORE, KB, 65], bf, tag="vg",
                              name="vg")
            for p in range(HEADS_PER_CORE // 2):
                qt_t = qk.tile([128, S], bf, tag=f"qt{p}", name=f"qt_{p}")
                kt_t = qk.tile([128, S], bf, tag=f"kt{p}", name=f"kt_{p}")
                nc.sync.dma_start(kt_t[:, :128], kt[p][:, :128])
                if p == 0:
                    nc.sync.dma_start(qt_t[:, :512], qt[p][:, :512])
                    nc.sync.dma_start(qt_t[:, 512:], qt[p][:, 512:])
                else:
                    nc.sync.dma_start(qt_t[:, :], qt[p][:, :])
                nc.scalar.dma_start(kt_t[:, 128:], kt[p][:, 128:])
                qt_ts.append(qt_t)
                kt_ts.append(kt_t)
            vg_src = vg.rearrange("h p k c -> p h k c")
            nc.gpsimd.dma_start(vg_t[:, 0:1, :, :], vg_src[:, 0:1, :, :])
            nc.gpsimd.dma_start(vg_t[:, 1:, :, :], vg_src[:, 1:, :, :])

            for p in range(HEADS_PER_CORE // 2):
                qt_t, kt_t = qt_ts[p], kt_ts[p]
                pts = ptsp.tile([128, 2, W], bf, tag="pts", name=f"pts_{p}")
                ots = [outp.tile([128, 16, 64], f32, tag=f"ot{s2}",
                                 name=f"ot_{p}_{s2}") for s2 in range(2)]
                accA = accp.tile([128, 7, 65], f32, tag="accA", name=f"accA_{p}")
                accB = accp.tile([128, 7, 65], f32, tag="accB", name=f"accB_{p}")
                accS = accp.tile([128, 7, 65], f32, tag="accS", name=f"accS_{p}")
                late = {}  # "a14": A qs14-15 + B qs0-4 tile; "b58": B qs5-8

                def pv(s2, kb, qs, acc_ap, start, stop):
                    h = 2 * p + s2
                    m = offs[kb] + 128 * (qs - kb)
                    nc.tensor.matmul(
                        acc_ap,
                        lhsT=pts[:, s2, m:m + 128],
                        rhs=vg_t[:, h, kb, :],
                        start=start, stop=stop,
                    )

                def a_acc(qs):
                    if qs < 7:
                        return accA[:, qs, :]
                    if qs < 14:
                        return accB[:, qs - 7, :]
                    return late["a14"][:, qs - 14, :]

                def emit_fill(kb, c, cw, cap, fi):
                    # one [A|B] fill of `cw` cols at strip-local col c, into a
                    # slot of per-head capacity `cap`; MMs split at 512-f32
                    # bank boundaries of the slot tile
                    q0 = 128 * kb
                    st = stp.tile([128, 2 * cap], f32, tag=f"st{cap}",
                                  name=f"st_{p}_{fi}")
                    for s2 in range(2):
                        po = 64 * s2
                        a = cap * s2
                        b = a + cw
                        cut = a
                        while cut < b:
                            nxt = min(b, (cut // 512 + 1) * 512)
                            nc.tensor.matmul(
                                st[:, cut:nxt],
                                lhsT=kt_t[po:po + 64, q0:q0 + 128],
                                rhs=qt_t[po:po + 64,
                                         q0 + c + cut - a:q0 + c + nxt - a],
                                start=True, stop=True,
                            )
                            cut = nxt
                    if MODE == "qk_only":
                        return
                    src = st.rearrange("r (s q) -> r s q", s=2)[:, :, 0:cw]
                    dst = pts[:, :, offs[kb] + c:offs[kb] + c + cw]
                    nc.scalar.activation(dst, src, Exp, scale=0.125)
                    if c == 0:
                        d = pts[:, :, offs[kb]:offs[kb] + 128]
                        nc.vector.tensor_mul(d, d, tri2[:, :, :])

                def emit_burst(kb):
                    # head A: open eager chains qs kb..13, plus qs14-15 once
                    # their bank exists (kb>=7); diag qs==kb last (tri dep)
                    qs_range = list(range(kb, 14))
                    qs_range += [qs for qs in (14, 15) if kb >= 7 and qs >= kb]
                    if kb > 0 and kb in qs_range:
                        qs_range.remove(kb)
                        qs_range.append(kb)
                    for qs in qs_range:
                        pv(0, kb, qs, a_acc(qs),
                           start=(kb == 0 and qs in (0, 7)),
                           stop=((qs == 6 and kb == 6)
                                 or (qs == 13 and kb == 13)
                                 or (qs == 15 and kb == 15)))
                    # head B eager: qs 9..15 in the accS bank
                    bq = [qs for qs in range(max(kb, 9), 16)]
                    if kb > 9 and bq and bq[0] == kb:
                        bq = bq[1:] + [kb]
                    for qs in bq:
                        pv(1, kb, qs, accS[:, qs - 9, :],
                           start=(kb == 0 and qs == 9),
                           stop=(qs == 15 and kb == 15))

                def norm(s2, acc_t, col0, qs_lo, n):
                    ot = ots[s2]
                    rs = small.tile([128, n], f32, tag="rs",
                                    name=f"rs5_{p}_{s2}_{qs_lo}")
                    nc.vector.reciprocal(rs[:, :], acc_t[:, col0:col0 + n, 64])
                    for j in range(n):
                        nc.vector.tensor_scalar_mul(
                            ot[:, qs_lo + j, :], acc_t[:, col0 + j, 0:64],
                            rs[:, j:j + 1])

                def dma_rows(s2, qs_lo, qs_hi):
                    h = 2 * p + s2
                    dst = o[h, qs_lo * 128:qs_hi * 128, :].rearrange(
                        "(j r) c -> r j c", r=128)
                    nc.sync.dma_start(dst, ots[s2][:, qs_lo:qs_hi, :])

                full = MODE == "full"

                def post_burst(j):
                    if j == 6:
                        norm(0, accA, 0, 0, 7)
                        dma_rows(0, 0, 7)
                        # reuse accA's bank: A qs14-15 (cols 0-1) + B qs0-4
                        # (cols 2-6).  Single start=True on the first write
                        # (A qs14 kb0) clears the bank; everything else relies
                        # on has_written overwrite-then-accumulate.
                        a14 = accp.tile([128, 7, 65], f32, tag="accA",
                                        name=f"a14_{p}")
                        late["a14"] = a14
                        for qs in (14, 15):
                            for kb2 in range(0, 7):
                                pv(0, kb2, qs, a14[:, qs - 14, :],
                                   start=(qs == 14 and kb2 == 0), stop=False)
                        for qs in range(0, 5):
                            for kb2 in range(0, qs + 1):
                                pv(1, kb2, qs, a14[:, 2 + qs, :],
                                   start=False, stop=False)
                        norm(1, a14, 2, 0, 5)
                        dma_rows(1, 0, 5)
                    if j == 13:
                        norm(0, accB, 0, 7, 7)
                        dma_rows(0, 7, 14)
                        # reuse accB's bank for B qs5-8
                        b58 = accp.tile([128, 4, 65], f32, tag="accB",
                                        name=f"b58_{p}")
                        late["b58"] = b58
                        for qs in range(5, 9):
                            for kb2 in range(0, qs + 1):
                                pv(1, kb2, qs, b58[:, qs - 5, :],
                                   start=(qs == 5 and kb2 == 0),
                                   stop=(qs == 8 and kb2 == qs))
                        norm(1, b58, 0, 5, 4)
                        dma_rows(1, 5, 9)
                    if j == 14:
                        norm(1, accS, 0, 9, 6)

                # fill loop: strict slot alternation (1536-slot first)
                fi = 0
                for kb in range(KB):
                    w = S - 128 * kb
                    c = 0
                    while c < w:
                        cap = 768 if fi % 2 == 0 else 512
                        cw = min(cap, w - c)
                        emit_fill(kb, c, cw, cap, fi)
                        fi += 1
                        c += cw
                    if full and kb >= 1:
                        emit_burst(kb - 1)
                        post_burst(kb - 1)
                if full:
                    emit_burst(KB - 1)
                    norm(0, late["a14"], 0, 14, 2)
                    dma_rows(0, 14, 16)
                    norm(1, accS, 6, 15, 1)
                    dma_rows(1, 9, 16)


def _emit(tc, nc, mybir, qt, kt, vg, tri, o, causal, reps=1):
    from contextlib import ExitStack

    f32 = mybir.dt.float32
    bf = mybir.dt.bfloat16
    Exp = mybir.ActivationFunctionType.Exp

    with ExitStack() as ctx:
        const = ctx.enter_context(tc.tile_pool(name="const", bufs=1))
        qk = ctx.enter_context(tc.tile_pool(name="qk", bufs=2))
        vpool = ctx.enter_context(tc.tile_pool(name="vp", bufs=2))
        ptp = ctx.enter_context(tc.tile_pool(name="ptp", bufs=PT_BUFS))
        outp = ctx.enter_context(tc.tile_pool(name="outp", bufs=4))
        small = ctx.enter_context(tc.tile_pool(name="small", bufs=4))
        stp = ctx.enter_context(tc.tile_pool(name="stp", bufs=ST_BUFS, space="PSUM"))
        accp = ctx.enter_context(tc.tile_pool(name="accp", bufs=1, space="PSUM"))

        tri_t = const.tile([128, 128], bf, name="tri_t")
        nc.sync.dma_start(tri_t[:, :], tri[:, :])

        # dummy exp issued first: walrus places the ~2.7us ACT table load
        # before the first ACTIVATE in the stream, so doing one on a tiny
        # constant tile overlaps the table load with the input DMAs instead
        # of serializing it before the first real exp
        warm = const.tile([128, 1], f32, name="warm")
        nc.vector.memset(warm[:, :], 0.0)
        nc.scalar.activation(warm[:, :], warm[:, :], Exp)

        from contextlib import nullcontext
        with (tc.For_i(0, reps, 1, hint_engines=(mybir.EngineType.PE,),
                       staggered_reset=STAGGERED)
              if reps > 1 else nullcontext()):
          rep = 0  # body emitted once; hardware loop repeats it
          # All inputs for the whole iteration are loaded up front, spread
          # across three DMA rings (SP + ACT HWDGE, gpsimd SWDGE) so nothing
          # downstream ever waits on a load except the very first strip:
          #   SP:  kt head-block for pair0 (tiny, unblocks QK(0) fast), qt
          #   ACT: kt bulk
          #   SWDGE: vg for all four heads (one batched start)
          qt_ts, kt_ts = [], []
          vg_t = vpool.tile([128, HEADS_PER_CORE, KB, 65], bf, tag="vg",
                            name=f"vg_{rep}")
          for p in range(HEADS_PER_CORE // 2):
              qt_t = qk.tile([128, S], bf, tag=f"qt{p}", name=f"qt_{rep}_{p}")
              kt_t = qk.tile([128, S], bf, tag=f"kt{p}", name=f"kt_{rep}_{p}")
              nc.sync.dma_start(kt_t[:, :128], kt[p][:, :128])
              nc.sync.dma_start(qt_t[:, :], qt[p][:, :])
              nc.scalar.dma_start(kt_t[:, 128:], kt[p][:, 128:])
              qt_ts.append(qt_t)
              kt_ts.append(kt_t)
          vg_src = vg.rearrange("h p k c -> p h k c")
          nc.gpsimd.dma_start(vg_t[:, 0:1, :, :], vg_src[:, 0:1, :, :])
          nc.gpsimd.dma_start(vg_t[:, 1:, :, :], vg_src[:, 1:, :, :])
          for p in range(HEADS_PER_CORE // 2):
              qt_t = qt_ts[p]
              kt_t = kt_ts[p]
              for s2 in range(2):
                  h = 2 * p + s2
                  po = 64 * s2  # partition offset of this head's d-dim

                  accA = accp.tile([128, 7, 65], f32, tag="accA", name=f"accA_{rep}_{h}")
                  accB = accp.tile([128, 7, 65], f32, tag="accB", name=f"accB_{rep}_{h}")
                  accC = accp.tile([128, 2, 65], f32, tag="accC", name=f"accC_{rep}_{h}")

                  def acc(i):
                      if i < 7:
                          return accA[:, i, :]
                      if i < 14:
                          return accB[:, i - 7, :]
                      return accC[:, i - 14, :]

                  def strip_halves(kb):
                      q0 = 128 * kb if causal else 0
                      cols = S - q0
                      pieces = []
                      hs = 0
                      while hs < cols:
                          pieces.append((q0, hs, min(1024, cols - hs), "A"))
                          hs += 1024
                      return pieces

                  def emit_qk(kb):
                      """QK matmuls for strip kb; returns the st tiles."""
                      sts = []
                      for q0, hs, hw, tg in strip_halves(kb):
                          st = stp.tile([128, 1024], f32, tag="st",
                                        name=f"st_{rep}_{h}_{kb}_{hs}")
                          for c0 in range(0, hw, 512):
                              cw = min(512, hw - c0)
                              nc.tensor.matmul(
                                  st[:, c0:c0 + cw],
                                  lhsT=kt_t[po:po + 64, kb * 128:(kb + 1) * 128],
                                  rhs=qt_t[po:po + 64,
                                           q0 + hs + c0:q0 + hs + c0 + cw],
                                  start=True, stop=True,
                              )
                          sts.append(st)
                      return sts

                  def normalize_bank(qs_lo, qs_hi):
                      """Normalize q_subs [qs_lo, qs_hi) and write out in one
                      batched DMA (rows r of out tile j map to q = qs*128+r)."""
                      n = qs_hi - qs_lo
                      ot = outp.tile([128, n, 64], f32, tag=f"ot{qs_lo}",
                                     name=f"ot_{rep}_{h}_{qs_lo}")
                      for j in range(n):
                          a = acc(qs_lo + j)
                          rs = small.tile([128, 1], f32, tag="rs",
                                          name=f"rs_{rep}_{h}_{qs_lo + j}")
                          nc.vector.reciprocal(rs[:, :], a[:, 64:65])
                          nc.vector.tensor_scalar_mul(ot[:, j, :], a[:, :64],
                                                      rs[:, :])
                      dst = o[h, qs_lo * 128:qs_hi * 128, :].rearrange(
                          "(j r) c -> r j c", r=128)
                      nc.sync.dma_start(dst, ot[:, :, :])

                  sts = emit_qk(0)
                  for kb in range(KB):
                      # exp of strip kb
                      pts = []
                      for (q0, hs, hw, tg), st in zip(strip_halves(kb), sts):
                          if MODE == "qk_only":
                              continue
                          pt = ptp.tile([128, 1024], bf, tag="pt",
                                        name=f"pt_{rep}_{h}_{kb}_{hs}")
                          nc.scalar.activation(pt[:, :hw], st[:, :hw], Exp,
                                               scale=0.125)
                          if causal and hs == 0:
                              nc.vector.tensor_mul(pt[:, :128], pt[:, :128],
                                                   tri_t[:, :])
                          pts.append(pt)
                      # QK for strip kb+1 goes to PE before PV of strip kb so
                      # the PE never stalls behind ACT
                      if kb + 1 < KB:
                          sts = emit_qk(kb + 1)
                      # PV accumulation for strip kb
                      if MODE != "full":
                          continue
                      for (q0, hs, hw, tg), pt in zip(strip_halves(kb), pts):
                          qs_range = list(range((q0 + hs) // 128,
                                                (q0 + hs + hw) // 128))
                          # the diagonal q_sub (== kb) additionally depends on
                          # the DVE tri-multiply; emit it last so the PE can
                          # start the other PV matmuls as soon as exp is done.
                          # (at kb==0 keep ascending order: the bank-group
                          # start=True matmuls must be first into each bank)
                          if causal and kb > 0 and qs_range and qs_range[0] == kb:
                              qs_range = qs_range[1:] + [kb]
                          for q_sub in qs_range:
                              m = q_sub * 128 - q0 - hs
                              last_kb = q_sub if causal else KB - 1
                              nc.tensor.matmul(
                                  acc(q_sub),
                                  lhsT=pt[:, m:m + 128],
                                  rhs=vg_t[:, h, kb, :],
                                  start=(kb == 0 and q_sub in _BANK_FIRST),
                                  stop=(q_sub in _BANK_LAST and kb == last_kb),
                              )
                      # normalize accumulator banks as soon as they complete
                      if causal:
                          if kb == 6:
                              normalize_bank(0, 7)
                          elif kb == 13:
                              normalize_bank(7, 14)
                          elif kb == 15:
                              normalize_bank(14, 16)
                  if not causal:
                      normalize_bank(0, 7)
                      normalize_bank(7, 14)
                      normalize_bank(14, 16)


def build_nc(causal=True, reps=1):
    """Build + compile the per-core Bass program (cached)."""
    key = ("nc", causal, reps, STRIP_W, MODE, ST_BUFS, PT_BUFS, STAGGERED,
           VERSION)
    if key in _built:
        return _built[key]
    import concourse.bacc as bacc
    from concourse import mybir, tile

    nc = bacc.Bacc("TRN2", target_bir_lowering=False, debug=False,
                   num_devices=N_CORES)
    qt = nc.dram_tensor("qt", (HEADS_PER_CORE // 2, 128, S),
                        mybir.dt.bfloat16, kind="ExternalInput").ap()
    kt = nc.dram_tensor("kt", (HEADS_PER_CORE // 2, 128, S),
                        mybir.dt.bfloat16, kind="ExternalInput").ap()
    vg = nc.dram_tensor("vg", (HEADS_PER_CORE, 128, KB, 65),
                        mybir.dt.bfloat16, kind="ExternalInput").ap()
    tri = nc.dram_tensor("tri", (128, 128), mybir.dt.bfloat16,
                         kind="ExternalInput").ap()
    o = nc.dram_tensor("o", (HEADS_PER_CORE, S, D), mybir.dt.float32,
                       kind="ExternalOutput").ap()
    with tile.TileContext(nc) as tc:
        if VERSION >= 5 and causal:
            _emit_v5(tc, nc, mybir, qt, kt, vg, tri, o, reps)
        elif VERSION >= 3 and causal:
            _emit_v3(tc, nc, mybir, qt, kt, vg, tri, o, reps)
        else:
            _emit(tc, nc, mybir, qt, kt, vg, tri, o, causal, reps)
    nc.compile()
    _built[key] = nc
    return nc


def prep_inputs(Q, K, V):
    """Host-side shard + layout prep. Returns list of 8 per-core input dicts."""
    Qf = np.ascontiguousarray(Q, dtype=np.float32).reshape(B * H, S, D)
    Kf = np.ascontiguousarray(K, dtype=np.float32).reshape(B * H, S, D)
    Vf = np.ascontiguousarray(V, dtype=np.float32).reshape(B * H, S, D)

    # [BH, S, D] -> transposed, bf16: [BH, D, S]
    Qt = np.ascontiguousarray(Qf.transpose(0, 2, 1)).astype(_BF16)
    Kt = np.ascontiguousarray(Kf.transpose(0, 2, 1)).astype(_BF16)

    # V augmented with ones column, partition-major: [BH, 128, KB, 65]
    Vb = Vf.astype(_BF16)
    vg_all = np.empty((B * H, 128, KB, 65), dtype=_BF16)
    # V[h, kb*128 + r, c] -> vg[h, r, kb, c]
    vg_all[:, :, :, :64] = Vb.reshape(B * H, KB, 128, D).transpose(0, 2, 1, 3)
    vg_all[:, :, :, 64] = _BF16(1.0)

    tri_np = (np.tril(np.ones((128, 128), dtype=np.float32))
              .T.astype(_BF16))  # tri[k, c] = 1 if c >= k
    tri_np = np.ascontiguousarray(tri_np)

    in_maps = []
    for c in range(N_CORES):
        h0 = c * HEADS_PER_CORE
        qt_c = np.empty((HEADS_PER_CORE // 2, 128, S), dtype=_BF16)
        kt_c = np.empty((HEADS_PER_CORE // 2, 128, S), dtype=_BF16)
        for p in range(HEADS_PER_CORE // 2):
            qt_c[p, :64] = Qt[h0 + 2 * p]
            qt_c[p, 64:] = Qt[h0 + 2 * p + 1]
            kt_c[p, :64] = Kt[h0 + 2 * p]
            kt_c[p, 64:] = Kt[h0 + 2 * p + 1]
        in_maps.append({
            "qt": qt_c,
            "kt": kt_c,
            "vg": np.ascontiguousarray(vg_all[h0:h0 + HEADS_PER_CORE]),
            "tri": tri_np,
        })
    return in_maps


def _classify_mask(mask):
    m = np.asarray(mask).reshape(S, S)
    if not m.any():
        return "dense"
    if np.array_equal(m, np.triu(np.ones((S, S), dtype=bool), k=1)):
        return "causal"
    raise NotImplementedError("only causal or all-False masks supported")


def run_cores(in_maps, causal=True, reps=1, **kwargs):
    from concourse import bass_utils

    nc = build_nc(causal, reps)
    return bass_utils.run_bass_kernel_spmd(
        nc, in_maps, core_ids=list(range(N_CORES)), **kwargs
    )


def kernel(Q, K, V, mask):
    kind = _classify_mask(mask)
    in_maps = prep_inputs(Q, K, V)
    res = run_cores(in_maps, causal=(kind == "causal"))
    out = np.concatenate([r["o"] for r in res.results], axis=0)
    return out.reshape(B, H, S, D).astype(np.float32)


if __name__ == "__main__":
    rng = np.random.default_rng(0)
    Q = rng.standard_normal((B, H, S, D), dtype=np.float32)
    K = rng.standard_normal((B, H, S, D), dtype=np.float32)
    V = rng.standard_normal((B, H, S, D), dtype=np.float32)
    mask = np.triu(np.ones((S, S), dtype=bool), k=1)[None, None]
    out = kernel(Q, K, V, mask)
    print("out", out.shape, out.dtype)



# revision 38
# speedup vs baseline: 1.0186x; 1.0118x over previous
"""Causal multi-head attention kernel for Trainium2 (8 NeuronCores).

Problem: B=2, H=16, S=2048, D=64 causal attention (softmax over last axis).
Sharding: 32 (batch, head) pairs split 4-per-core across 8 cores; each core
computes its heads independently (no collectives).

v3 per-core algorithm (pair-interleaved, S-transposed layout; see _emit_v3):
  - Host pre-packs qt/kt with the two heads of a pair on SBUF partitions
    0-63 / 64-127.  Their K=64 QK matmuls auto-derive PE tile_position
    (0,0)/(64,0) from base partitions, so adjacent A/B matmuls run
    CONCURRENTLY on the PE row-groups (measured exactly 2x).
  - Each 512-col fill writes [A|B] halves of one [128,1024] f32 PSUM strip
    slot (2 slots, 4 banks), and ONE joint ACTIVATE exps BOTH heads
    (FD=1024) into a per-pair P^T store in SBUF.  ACT per-instruction
    overhead is ~430 ns on HW, so instruction count dominates the exp cost;
    the joint exps cut it from 96 to 80 per core.
  - PV kb-outer: head A's 16 chains eager in 3 PSUM banks (7/7/2 + ones-col
    row-sum trick); head B's qs9-15 chains eager in the spare 8th bank;
    B's qs0-6 / qs7-8 chains reuse A's banks as A's bank-group
    normalizations release them.  Every bank group is normalized (batched
    DVE reciprocal + per-row scalar mul) and DMA'd out the moment it closes.
  - Input DMAs are batched per iteration and spread across the SP HWDGE,
    ACT HWDGE, and gpsimd SWDGE rings.

Measured per-iteration HW time (8-core SPMD, repeat-differenced): ~94 us
vs 127-134 us for the per-head sequential baseline (VERSION=2 path).

kernel(Q, K, V, mask) takes the full unsharded fp32 inputs and returns the
full [2, 16, 2048, 64] fp32 output.
"""

import sys

if "/opt/trn_rl_repo" not in sys.path:
    sys.path.insert(0, "/opt/trn_rl_repo")

import numpy as np
import ml_dtypes

B, H, S, D = 2, 16, 2048, 64
N_CORES = 8
HEADS_PER_CORE = (B * H) // N_CORES  # 4
KB = S // 128  # 16 k-blocks per head
QS = S // 128  # 16 q-subblocks per head

_BF16 = ml_dtypes.bfloat16

# accumulator bank packing: q_subs 0-6 -> bank A, 7-13 -> bank B, 14-15 -> C
_BANK_FIRST = (0, 7, 14)   # first q_sub written in each accumulator bank
_BANK_LAST = (6, 13, 15)   # last q_sub written in each accumulator bank

_built = {}
STRIP_W = 1024
MODE = "full"  # full | qk_only | qk_exp (timing ablations)
ST_BUFS = 2  # PSUM strip-tile slots  # S^T strip tile width (PSUM free elems)
PT_BUFS = 4   # SBUF P^T tile slots (exp outputs)
STAGGERED = True  # staggered_reset on the timing loop (overlaps iterations)
VERSION = 3   # 2 = per-head sequential; 3 = pair-interleaved row-tiled (BEST)
              # 4 = v3 + packed fills (SLOWER: PE LDW thrash > ACT savings)
              # 5 = v3 + 3-bank accs -> 5 strip banks, 1536/1024 slots
              #     (FAULTS on HW: A/B row-tiled QK matmuls of a 1536-wide
              #      slot land concurrently in the same PSUM bank)


def _emit_v3(tc, nc, mybir, qt, kt, vg, tri, o, reps=1):
    """Pair-interleaved causal attention.

    The two heads of a pair occupy SBUF partitions 0-63 (A) and 64-127 (B) of
    qt/kt.  Their QK matmuls carry tile_position (0,0)/(64,0) automatically
    (base partitions), so adjacent A/B matmuls run CONCURRENTLY on the PE's
    row-groups.  Each 512-col fill writes [A|B] side by side into one
    [128,1024] PSUM strip slot, and ONE joint ACTIVATE exps both heads into a
    per-pair P^T store in SBUF (pts).  PV for head A runs kb-outer eagerly
    (accs in 3 banks); head B's qs7-13 chains run eagerly in the spare 8th
    bank, and its qs0-6 / qs14-15 chains reuse A's banks as A's
    normalizations release them.  PSUM: 2x2-bank strip slots + 4 acc banks =
    exactly 8.
    """
    from contextlib import ExitStack, nullcontext

    f32 = mybir.dt.float32
    bf = mybir.dt.bfloat16
    Exp = mybir.ActivationFunctionType.Exp

    offs = []
    off = 0
    for kb in range(KB):
        offs.append(off)
        off += S - 128 * kb
    W = off  # 17408

    with ExitStack() as ctx:
        const = ctx.enter_context(tc.tile_pool(name="const", bufs=1))
        qk = ctx.enter_context(tc.tile_pool(name="qk", bufs=1))
        vpool = ctx.enter_context(tc.tile_pool(name="vp", bufs=1))
        ptsp = ctx.enter_context(tc.tile_pool(name="ptsp", bufs=2))
        outp = ctx.enter_context(tc.tile_pool(name="outp", bufs=2))
        small = ctx.enter_context(tc.tile_pool(name="small", bufs=4))
        stp = ctx.enter_context(tc.tile_pool(name="stp", bufs=ST_BUFS, space="PSUM"))
        accp = ctx.enter_context(tc.tile_pool(name="accp", bufs=1, space="PSUM"))

        tri2 = const.tile([128, 2, 128], bf, name="tri2")
        nc.gpsimd.dma_start(tri2[:, 0, :], tri[:, :])
        nc.gpsimd.dma_start(tri2[:, 1, :], tri[:, :])

        # dummy exp: hoists the ~2.7us ACT table load ahead of the input DMAs
        warm = const.tile([128, 1], f32, name="warm")
        nc.vector.memset(warm[:, :], 0.0)
        nc.scalar.activation(warm[:, :], warm[:, :], Exp)

        with (tc.For_i(0, reps, 1, hint_engines=(mybir.EngineType.PE,),
                       staggered_reset=STAGGERED)
              if reps > 1 else nullcontext()):
            # iteration-wide input loads, spread over SP/ACT/SWDGE rings
            qt_ts, kt_ts = [], []
            vg_t = vpool.tile([128, HEADS_PER_CORE, KB, 65], bf, tag="vg",
                              name="vg")
            for p in range(HEADS_PER_CORE // 2):
                qt_t = qk.tile([128, S], bf, tag=f"qt{p}", name=f"qt_{p}")
                kt_t = qk.tile([128, S], bf, tag=f"kt{p}", name=f"kt_{p}")
                nc.sync.dma_start(kt_t[:, :128], kt[p][:, :128])
                if p == 0:
                    nc.sync.dma_start(qt_t[:, :512], qt[p][:, :512])
                    nc.sync.dma_start(qt_t[:, 512:], qt[p][:, 512:])
                else:
                    nc.sync.dma_start(qt_t[:, :], qt[p][:, :])
                nc.scalar.dma_start(kt_t[:, 128:], kt[p][:, 128:])
                qt_ts.append(qt_t)
                kt_ts.append(kt_t)
            vg_src = vg.rearrange("h p k c -> p h k c")
            nc.gpsimd.dma_start(vg_t[:, 0:1, :, :], vg_src[:, 0:1, :, :])
            nc.gpsimd.dma_start(vg_t[:, 1:, :, :], vg_src[:, 1:, :, :])

            for p in range(HEADS_PER_CORE // 2):
                qt_t, kt_t = qt_ts[p], kt_ts[p]
                pts = ptsp.tile([128, 2, W], bf, tag="pts", name=f"pts_{p}")
                ots = [outp.tile([128, 16, 64], f32, tag=f"ot{s2}",
                                 name=f"ot_{p}_{s2}") for s2 in range(2)]
                accA = accp.tile([128, 7, 65], f32, tag="accA", name=f"accA_{p}")
                accB = accp.tile([128, 7, 65], f32, tag="accB", name=f"accB_{p}")
                accC = accp.tile([128, 2, 65], f32, tag="accC", name=f"accC_{p}")
                accS = accp.tile([128, 7, 65], f32, tag="accS", name=f"accS_{p}")

                def accs_for(s2, qs, _t=(accA, accB, accC, accS)):
                    aA, aB, aC, aS = _t
                    if s2 == 0:
                        if qs < 7:
                            return aA[:, qs, :]
                        if qs < 14:
                            return aB[:, qs - 7, :]
                        return aC[:, qs - 14, :]
                    assert 9 <= qs < 16
                    return aS[:, qs - 9, :]

                def emit_fill_v3(kb, c):
                    w = S - 128 * kb
                    q0 = 128 * kb
                    cw = min(512, w - c)
                    st = stp.tile([128, 1024], f32, tag="st",
                                  name=f"st_{p}_{kb}_{c}")
                    for s2 in range(2):
                        po = 64 * s2
                        nc.tensor.matmul(
                            st[:, 512 * s2:512 * s2 + cw],
                            lhsT=kt_t[po:po + 64, q0:q0 + 128],
                            rhs=qt_t[po:po + 64, q0 + c:q0 + c + cw],
                            start=True, stop=True,
                        )
                    if MODE == "qk_only":
                        return
                    src = st.rearrange("r (s q) -> r s q", s=2)[:, :, 0:cw]
                    dst = pts[:, :, offs[kb] + c:offs[kb] + c + cw]
                    nc.scalar.activation(dst, src, Exp, scale=0.125)
                    if c == 0:
                        d = pts[:, :, offs[kb]:offs[kb] + 128]
                        nc.vector.tensor_mul(d, d, tri2[:, :, :])

                def emit_fill_1415():
                    """Merged fill for the two tiny tail strips (kb=14: 256
                    cols, kb=15: 128 cols) — one slot, one joint ACTIVATE
                    (FD 768) instead of two."""
                    st = stp.tile([128, 1024], f32, tag="st",
                                  name=f"st_{p}_1415")
                    for s2 in range(2):
                        po = 64 * s2
                        nc.tensor.matmul(
                            st[:, 512 * s2:512 * s2 + 256],
                            lhsT=kt_t[po:po + 64, 14 * 128:15 * 128],
                            rhs=qt_t[po:po + 64, 14 * 128:14 * 128 + 256],
                            start=True, stop=True,
                        )
                        nc.tensor.matmul(
                            st[:, 512 * s2 + 256:512 * s2 + 384],
                            lhsT=kt_t[po:po + 64, 15 * 128:16 * 128],
                            rhs=qt_t[po:po + 64, 15 * 128:15 * 128 + 128],
                            start=True, stop=True,
                        )
                    if MODE == "qk_only":
                        return
                    src = st.rearrange("r (s q) -> r s q", s=2)[:, :, 0:384]
                    dst = pts[:, :, offs[14]:offs[14] + 384]
                    nc.scalar.activation(dst, src, Exp, scale=0.125)
                    for kb in (14, 15):
                        d = pts[:, :, offs[kb]:offs[kb] + 128]
                        nc.vector.tensor_mul(d, d, tri2[:, :, :])

                def emit_fill_v4(fi):
                    """Packed fill fi: stream cols [512*fi, 512*(fi+1)) of the
                    concatenated causal strips; may span strip boundaries."""
                    lo, hi = 512 * fi, 512 * (fi + 1)
                    st = stp.tile([128, 1024], f32, tag="st",
                                  name=f"st_{p}_{fi}")
                    segs = []
                    for kb in range(KB):
                        a = max(lo, offs[kb])
                        b = min(hi, offs[kb] + (S - 128 * kb))
                        if a < b:
                            segs.append((kb, a, b))
                    for kb, a, b in segs:
                        q0 = 128 * kb
                        for s2 in range(2):
                            po = 64 * s2
                            nc.tensor.matmul(
                                st[:, 512 * s2 + a - lo:512 * s2 + b - lo],
                                lhsT=kt_t[po:po + 64, q0:q0 + 128],
                                rhs=qt_t[po:po + 64,
                                         q0 + a - offs[kb]:q0 + b - offs[kb]],
                                start=True, stop=True,
                            )
                    if MODE == "qk_only":
                        return
                    src = st.rearrange("r (s q) -> r s q", s=2)[:, :, :]
                    nc.scalar.activation(pts[:, :, lo:hi], src, Exp, scale=0.125)
                    for kb, a, b in segs:
                        # tri-mask the diag block once its last column is exp'd
                        dend = offs[kb] + 128
                        if a <= dend - 1 < b:
                            d = pts[:, :, offs[kb]:dend]
                            nc.vector.tensor_mul(d, d, tri2[:, :, :])

                def pv(s2, kb, qs, acc_ap, start, stop):
                    h = 2 * p + s2
                    m = offs[kb] + 128 * (qs - kb)
                    nc.tensor.matmul(
                        acc_ap,
                        lhsT=pts[:, s2, m:m + 128],
                        rhs=vg_t[:, h, kb, :],
                        start=start, stop=stop,
                    )

                def emit_burst(kb):
                    # head A: all open chains; diag qs==kb last (tri dep)
                    qs_range = list(range(kb, 16))
                    if kb > 0:
                        qs_range = qs_range[1:] + [kb]
                    for qs in qs_range:
                        pv(0, kb, qs, accs_for(0, qs),
                           start=(kb == 0 and qs in (0, 7, 14)),
                           stop=(qs in (6, 13, 15) and kb == qs))
                    # head B eager subset: qs 9..15 in the spare bank
                    bq = [qs for qs in range(max(kb, 9), 16)]
                    if kb > 9 and bq and bq[0] == kb:
                        bq = bq[1:] + [kb]
                    for qs in bq:
                        pv(1, kb, qs, accs_for(1, qs),
                           start=(kb == 0 and qs == 9),
                           stop=(qs == 15 and kb == qs))

                def norm(s2, acc_t, col0, qs_lo, n):
                    ot = ots[s2]
                    rs = small.tile([128, n], f32, tag="rs",
                                    name=f"rs_{p}_{s2}_{qs_lo}")
                    nc.vector.reciprocal(rs[:, :], acc_t[:, col0:col0 + n, 64])
                    for j in range(n):
                        nc.vector.tensor_scalar_mul(
                            ot[:, qs_lo + j, :], acc_t[:, col0 + j, 0:64],
                            rs[:, j:j + 1])

                def dma_rows(s2, qs_lo, qs_hi):
                    h = 2 * p + s2
                    dst = o[h, qs_lo * 128:qs_hi * 128, :].rearrange(
                        "(j r) c -> r j c", r=128)
                    nc.sync.dma_start(dst, ots[s2][:, qs_lo:qs_hi, :])

                full = MODE == "full"

                def post_burst(j):
                    # bank-group completions: normalize + store the moment a
                    # group's last chain closes, so nothing piles up at the end
                    if j == 6:
                        norm(0, accA, 0, 0, 7)
                        dma_rows(0, 0, 7)
                        # B's qs0-6 chains into A's freed bank
                        accA2 = accp.tile([128, 7, 65], f32, tag="accA",
                                          name=f"accA2_{p}")
                        for kb2 in range(0, 7):
                            for qs in range(kb2, 7):
                                pv(1, kb2, qs, accA2[:, qs, :],
                                   start=(kb2 == 0 and qs == 0),
                                   stop=(qs == 6 and kb2 == qs))
                        norm(1, accA2, 0, 0, 7)
                        dma_rows(1, 0, 7)
                    if j == 8:
                        # B's qs7-8 chains: reuse the accA bank a second time
                        # (accA2 was normalized and released at j==6+), so
                        # these 17 MMs run mid-stream instead of on the tail
                        accC2 = accp.tile([128, 2, 65], f32, tag="accA",
                                          name=f"accC2_{p}")
                        for kb2 in range(0, 9):
                            for qs in (7, 8):
                                if qs < kb2:
                                    continue
                                pv(1, kb2, qs, accC2[:, qs - 7, :],
                                   start=(kb2 == 0 and qs == 7),
                                   stop=(qs == 8 and kb2 == qs))
                        norm(1, accC2, 0, 7, 2)
                        dma_rows(1, 7, 9)
                    if j == 13:
                        norm(0, accB, 0, 7, 7)
                        dma_rows(0, 7, 14)
                    if j == 14:
                        # B's accS chains qs9-14 are closed (cols 0..5)
                        norm(1, accS, 0, 9, 6)

                if VERSION >= 4:
                    end_fill = {}
                    for kb in range(KB):
                        e = (offs[kb + 1] - 1 if kb + 1 < KB else W - 1) // 512
                        end_fill.setdefault(e, []).append(kb)
                    for fi in range(W // 512):
                        emit_fill_v4(fi)
                        if not full:
                            continue
                        for kb in end_fill.get(fi, []):
                            if kb >= 1:
                                emit_burst(kb - 1)
                                post_burst(kb - 1)
                else:
                    for kb in range(KB):
                        if kb == 15:
                            continue  # folded into the kb==14 merged fill
                        if kb == 14:
                            emit_fill_1415()
                        else:
                            for c in range(0, S - 128 * kb, 512):
                                emit_fill_v3(kb, c)
                        if full and kb >= 1:
                            emit_burst(kb - 1)
                            post_burst(kb - 1)
                        if full and kb == 14:
                            emit_burst(14)
                            post_burst(14)
                if full:
                    emit_burst(KB - 1)
                    norm(0, accC, 0, 14, 2)
                    dma_rows(0, 14, 16)
                    norm(1, accS, 6, 15, 1)
                    dma_rows(1, 9, 16)


def _emit_v5(tc, nc, mybir, qt, kt, vg, tri, o, reps=1):
    """v3 + 3-bank accumulator choreography freeing a 5th strip bank.

    Strip slots become an asymmetric ping-pong pair [128,1536]+[128,1024]
    ([A768|B768] / [A512|B512] fills), cutting joint-exp ACTIVATEs from 40
    to ~30 per head-pair (ACT per-instruction overhead is ~430 ns, so this
    is ~4.3 us/core of ACT busy).  Accumulator banks:
      bank accA: t1 = A qs0-6 eager; t2 (after kb6) = A qs14-15 + B qs0-4
      bank accB: t1 = A qs7-13 eager; t2 (after kb13) = B qs5-8
      bank accS: B qs9-15 eager
    A qs14-15 prefix chains (kb0-6) catch up in one block at kb6, then ride
    the regular bursts; single-start-per-bank ordering is preserved.
    """
    from contextlib import ExitStack, nullcontext

    f32 = mybir.dt.float32
    bf = mybir.dt.bfloat16
    Exp = mybir.ActivationFunctionType.Exp

    offs = []
    off = 0
    for kb in range(KB):
        offs.append(off)
        off += S - 128 * kb
    W = off  # 17408

    with ExitStack() as ctx:
        const = ctx.enter_context(tc.tile_pool(name="const", bufs=1))
        qk = ctx.enter_context(tc.tile_pool(name="qk", bufs=1))
        vpool = ctx.enter_context(tc.tile_pool(name="vp", bufs=1))
        ptsp = ctx.enter_context(tc.tile_pool(name="ptsp", bufs=2))
        outp = ctx.enter_context(tc.tile_pool(name="outp", bufs=2))
        small = ctx.enter_context(tc.tile_pool(name="small", bufs=4))
        stp = ctx.enter_context(tc.tile_pool(name="stp", bufs=1, space="PSUM"))
        accp = ctx.enter_context(tc.tile_pool(name="accp", bufs=1, space="PSUM"))

        tri2 = const.tile([128, 2, 128], bf, name="tri2")
        nc.gpsimd.dma_start(tri2[:, 0, :], tri[:, :])
        nc.gpsimd.dma_start(tri2[:, 1, :], tri[:, :])

        warm = const.tile([128, 1], f32, name="warm")
        nc.vector.memset(warm[:, :], 0.0)
        nc.scalar.activation(warm[:, :], warm[:, :], Exp)

        with (tc.For_i(0, reps, 1, hint_engines=(mybir.EngineType.PE,),
                       staggered_reset=STAGGERED)
              if reps > 1 else nullcontext()):
            qt_ts, kt_ts = [], []
            vg_t = vpool.tile([128, HEADS_PER_CORE, KB, 65], bf, tag="vg",
                              name="vg")
            for p in range(HEADS_PER_CORE // 2):
                qt_t = qk.tile([128, S], bf, tag=f"qt{p}", name=f"qt_{p}")
                kt_t = qk.tile([128, S], bf, tag=f"kt{p}", name=f"kt_{p}")
                nc.sync.dma_start(kt_t[:, :128], kt[p][:, :128])
                if p == 0:
                    nc.sync.dma_start(qt_t[:, :512], qt[p][:, :512])
                    nc.sync.dma_start(qt_t[:, 512:], qt[p][:, 512:])
                else:
                    nc.sync.dma_start(qt_t[:, :], qt[p][:, :])
                nc.scalar.dma_start(kt_t[:, 128:], kt[p][:, 128:])
                qt_ts.append(qt_t)
                kt_ts.append(kt_t)
            vg_src = vg.rearrange("h p k c -> p h k c")
            nc.gpsimd.dma_start(vg_t[:, 0:1, :, :], vg_src[:, 0:1, :, :])
            nc.gpsimd.dma_start(vg_t[:, 1:, :, :], vg_src[:, 1:, :, :])

            for p in range(HEADS_PER_CORE // 2):
                qt_t, kt_t = qt_ts[p], kt_ts[p]
                pts = ptsp.tile([128, 2, W], bf, tag="pts", name=f"pts_{p}")
                ots = [outp.tile([128, 16, 64], f32, tag=f"ot{s2}",
                                 name=f"ot_{p}_{s2}") for s2 in range(2)]
                accA = accp.tile([128, 7, 65], f32, tag="accA", name=f"accA_{p}")
                accB = accp.tile([128, 7, 65], f32, tag="accB", name=f"accB_{p}")
                accS = accp.tile([128, 7, 65], f32, tag="accS", name=f"accS_{p}")
                late = {}  # "a14": A qs14-15 + B qs0-4 tile; "b58": B qs5-8

                def pv(s2, kb, qs, acc_ap, start, stop):
                    h = 2 * p + s2
                    m = offs[kb] + 128 * (qs - kb)
                    nc.tensor.matmul(
                        acc_ap,
                        lhsT=pts[:, s2, m:m + 128],
                        rhs=vg_t[:, h, kb, :],
                        start=start, stop=stop,
                    )

                def a_acc(qs):
                    if qs < 7:
                        return accA[:, qs, :]
                    if qs < 14:
                        return accB[:, qs - 7, :]
                    return late["a14"][:, qs - 14, :]

                def emit_fill(kb, c, cw, cap, fi):
                    # one [A|B] fill of `cw` cols at strip-local col c, into a
                    # slot of per-head capacity `cap`; MMs split at 512-f32
                    # bank boundaries of the slot tile
                    q0 = 128 * kb
                    st = stp.tile([128, 2 * cap], f32, tag=f"st{cap}",
                                  name=f"st_{p}_{fi}")
                    for s2 in range(2):
                        po = 64 * s2
                        a = cap * s2
                        b = a + cw
                        cut = a
                        while cut < b:
                            nxt = min(b, (cut // 512 + 1) * 512)
                            nc.tensor.matmul(
                                st[:, cut:nxt],
                                lhsT=kt_t[po:po + 64, q0:q0 + 128],
                                rhs=qt_t[po:po + 64,
                                         q0 + c + cut - a:q0 + c + nxt - a],
                                start=True, stop=True,
                            )
                            cut = nxt
                    if MODE == "qk_only":
                        return
                    src = st.rearrange("r (s q) -> r s q", s=2)[:, :, 0:cw]
                    dst = pts[:, :, offs[kb] + c:offs[kb] + c + cw]
                    nc.scalar.activation(dst, src, Exp, scale=0.125)
                    if c == 0:
                        d = pts[:, :, offs[kb]:offs[kb] + 128]
                        nc.vector.tensor_mul(d, d, tri2[:, :, :])

                def emit_burst(kb):
                    # head A: open eager chains qs kb..13, plus qs14-15 once
                    # their bank exists (kb>=7); diag qs==kb last (tri dep)
                    qs_range = list(range(kb, 14))
                    qs_range += [qs for qs in (14, 15) if kb >= 7 and qs >= kb]
                    if kb > 0 and kb in qs_range:
                        qs_range.remove(kb)
                        qs_range.append(kb)
                    for qs in qs_range:
                        pv(0, kb, qs, a_acc(qs),
                           start=(kb == 0 and qs in (0, 7)),
                           stop=((qs == 6 and kb == 6)
                                 or (qs == 13 and kb == 13)
                                 or (qs == 15 and kb == 15)))
                    # head B eager: qs 9..15 in the accS bank
                    bq = [qs for qs in range(max(kb, 9), 16)]
                    if kb > 9 and bq and bq[0] == kb:
                        bq = bq[1:] + [kb]
                    for qs in bq:
                        pv(1, kb, qs, accS[:, qs - 9, :],
                           start=(kb == 0 and qs == 9),
                           stop=(qs == 15 and kb == 15))

                def norm(s2, acc_t, col0, qs_lo, n):
                    ot = ots[s2]
                    rs = small.tile([128, n], f32, tag="rs",
                                    name=f"rs5_{p}_{s2}_{qs_lo}")
                    nc.vector.reciprocal(rs[:, :], acc_t[:, col0:col0 + n, 64])
                    for j in range(n):
                        nc.vector.tensor_scalar_mul(
                            ot[:, qs_lo + j, :], acc_t[:, col0 + j, 0:64],
                            rs[:, j:j + 1])

                def dma_rows(s2, qs_lo, qs_hi):
                    h = 2 * p + s2
                    dst = o[h, qs_lo * 128:qs_hi * 128, :].rearrange(
                        "(j r) c -> r j c", r=128)
                    nc.sync.dma_start(dst, ots[s2][:, qs_lo:qs_hi, :])

                full = MODE == "full"

                def post_burst(j):
                    if j == 6:
                        norm(0, accA, 0, 0, 7)
                        dma_rows(0, 0, 7)
                        # reuse accA's bank: A qs14-15 (cols 0-1) + B qs0-4
                        # (cols 2-6).  Single start=True on the first write
                        # (A qs14 kb0) clears the bank; everything else relies
                        # on has_written overwrite-then-accumulate.
                        a14 = accp.tile([128, 7, 65], f32, tag="accA",
                                        name=f"a14_{p}")
                        late["a14"] = a14
                        for qs in (14, 15):
                            for kb2 in range(0, 7):
                                pv(0, kb2, qs, a14[:, qs - 14, :],
                                   start=(qs == 14 and kb2 == 0), stop=False)
                        for qs in range(0, 5):
                            for kb2 in range(0, qs + 1):
                                pv(1, kb2, qs, a14[:, 2 + qs, :],
                                   start=False, stop=False)
                        norm(1, a14, 2, 0, 5)
                        dma_rows(1, 0, 5)
                    if j == 13:
                        norm(0, accB, 0, 7, 7)
                        dma_rows(0, 7, 14)
                        # reuse accB's bank for B qs5-8
                        b58 = accp.tile([128, 4, 65], f32, tag="accB",
                                        name=f"b58_{p}")
                        late["b58"] = b58
                        for qs in range(5, 9):
                            for kb2 in range(0, qs + 1):
                                pv(1, kb2, qs, b58[:, qs - 5, :],
                                   start=(qs == 5 and kb2 == 0),
                                   stop=(qs == 8 and kb2 == qs))
                        norm(1, b58, 0, 5, 4)
                        dma_rows(1, 5, 9)
                    if j == 14:
                        norm(1, accS, 0, 9, 6)

                # fill loop: strict slot alternation (1536-slot first)
                fi = 0
                for kb in range(KB):
                    w = S - 128 * kb
                    c = 0
                    while c < w:
                        cap = 768 if fi % 2 == 0 else 512
                        cw = min(cap, w - c)
                        emit_fill(kb, c, cw, cap, fi)
                        fi += 1
                        c += cw
                    if full and kb >= 1:
                        emit_burst(kb - 1)
                        post_burst(kb - 1)
                if full:
                    emit_burst(KB - 1)
                    norm(0, late["a14"], 0, 14, 2)
                    dma_rows(0, 14, 16)
                    norm(1, accS, 6, 15, 1)
                    dma_rows(1, 9, 16)


def _emit(tc, nc, mybir, qt, kt, vg, tri, o, causal, reps=1):
    from contextlib import ExitStack

    f32 = mybir.dt.float32
    bf = mybir.dt.bfloat16
    Exp = mybir.ActivationFunctionType.Exp

    with ExitStack() as ctx:
        const = ctx.enter_context(tc.tile_pool(name="const", bufs=1))
        qk = ctx.enter_context(tc.tile_pool(name="qk", bufs=2))
        vpool = ctx.enter_context(tc.tile_pool(name="vp", bufs=2))
        ptp = ctx.enter_context(tc.tile_pool(name="ptp", bufs=PT_BUFS))
        outp = ctx.enter_context(tc.tile_pool(name="outp", bufs=4))
        small = ctx.enter_context(tc.tile_pool(name="small", bufs=4))
        stp = ctx.enter_context(tc.tile_pool(name="stp", bufs=ST_BUFS, space="PSUM"))
        accp = ctx.enter_context(tc.tile_pool(name="accp", bufs=1, space="PSUM"))

        tri_t = const.tile([128, 128], bf, name="tri_t")
        nc.sync.dma_start(tri_t[:, :], tri[:, :])

        # dummy exp issued first: walrus places the ~2.7us ACT table load
        # before the first ACTIVATE in the stream, so doing one on a tiny
        # constant tile overlaps the table load with the input DMAs instead
        # of serializing it before the first real exp
        warm = const.tile([128, 1], f32, name="warm")
        nc.vector.memset(warm[:, :], 0.0)
        nc.scalar.activation(warm[:, :], warm[:, :], Exp)

        from contextlib import nullcontext
        with (tc.For_i(0, reps, 1, hint_engines=(mybir.EngineType.PE,),
                       staggered_reset=STAGGERED)
              if reps > 1 else nullcontext()):
          rep = 0  # body emitted once; hardware loop repeats it
          # All inputs for the whole iteration are loaded up front, spread
          # across three DMA rings (SP + ACT HWDGE, gpsimd SWDGE) so nothing
          # downstream ever waits on a load except the very first strip:
          #   SP:  kt head-block for pair0 (tiny, unblocks QK(0) fast), qt
          #   ACT: kt bulk
          #   SWDGE: vg for all four heads (one batched start)
          qt_ts, kt_ts = [], []
          vg_t = vpool.tile([128, HEADS_PER_CORE, KB, 65], bf, tag="vg",
                            name=f"vg_{rep}")
          for p in range(HEADS_PER_CORE // 2):
              qt_t = qk.tile([128, S], bf, tag=f"qt{p}", name=f"qt_{rep}_{p}")
              kt_t = qk.tile([128, S], bf, tag=f"kt{p}", name=f"kt_{rep}_{p}")
              nc.sync.dma_start(kt_t[:, :128], kt[p][:, :128])
              nc.sync.dma_start(qt_t[:, :], qt[p][:, :])
              nc.scalar.dma_start(kt_t[:, 128:], kt[p][:, 128:])
              qt_ts.append(qt_t)
              kt_ts.append(kt_t)
          vg_src = vg.rearrange("h p k c -> p h k c")
          nc.gpsimd.dma_start(vg_t[:, 0:1, :, :], vg_src[:, 0:1, :, :])
          nc.gpsimd.dma_start(vg_t[:, 1:, :, :], vg_src[:, 1:, :, :])
          for p in range(HEADS_PER_CORE // 2):
              qt_t = qt_ts[p]
              kt_t = kt_ts[p]
              for s2 in range(2):
                  h = 2 * p + s2
                  po = 64 * s2  # partition offset of this head's d-dim

                  accA = accp.tile([128, 7, 65], f32, tag="accA", name=f"accA_{rep}_{h}")
                  accB = accp.tile([128, 7, 65], f32, tag="accB", name=f"accB_{rep}_{h}")
                  accC = accp.tile([128, 2, 65], f32, tag="accC", name=f"accC_{rep}_{h}")

                  def acc(i):
                      if i < 7:
                          return accA[:, i, :]
                      if i < 14:
                          return accB[:, i - 7, :]
                      return accC[:, i - 14, :]

                  def strip_halves(kb):
                      q0 = 128 * kb if causal else 0
                      cols = S - q0
                      pieces = []
                      hs = 0
                      while hs < cols:
                          pieces.append((q0, hs, min(1024, cols - hs), "A"))
                          hs += 1024
                      return pieces

                  def emit_qk(kb):
                      """QK matmuls for strip kb; returns the st tiles."""
                      sts = []
                      for q0, hs, hw, tg in strip_halves(kb):
                          st = stp.tile([128, 1024], f32, tag="st",
                                        name=f"st_{rep}_{h}_{kb}_{hs}")
                          for c0 in range(0, hw, 512):
                              cw = min(512, hw - c0)
                              nc.tensor.matmul(
                                  st[:, c0:c0 + cw],
                                  lhsT=kt_t[po:po + 64, kb * 128:(kb + 1) * 128],
                                  rhs=qt_t[po:po + 64,
                                           q0 + hs + c0:q0 + hs + c0 + cw],
                                  start=True, stop=True,
                              )
                          sts.append(st)
                      return sts

                  def normalize_bank(qs_lo, qs_hi):
                      """Normalize q_subs [qs_lo, qs_hi) and write out in one
                      batched DMA (rows r of out tile j map to q = qs*128+r)."""
                      n = qs_hi - qs_lo
                      ot = outp.tile([128, n, 64], f32, tag=f"ot{qs_lo}",
                                     name=f"ot_{rep}_{h}_{qs_lo}")
                      for j in range(n):
                          a = acc(qs_lo + j)
                          rs = small.tile([128, 1], f32, tag="rs",
                                          name=f"rs_{rep}_{h}_{qs_lo + j}")
                          nc.vector.reciprocal(rs[:, :], a[:, 64:65])
                          nc.vector.tensor_scalar_mul(ot[:, j, :], a[:, :64],
                                                      rs[:, :])
                      dst = o[h, qs_lo * 128:qs_hi * 128, :].rearrange(
                          "(j r) c -> r j c", r=128)
                      nc.sync.dma_start(dst, ot[:, :, :])

                  sts = emit_qk(0)
                  for kb in range(KB):
                      # exp of strip kb
                      pts = []
                      for (q0, hs, hw, tg), st in zip(strip_halves(kb), sts):
                          if MODE == "qk_only":
                              continue
                          pt = ptp.tile([128, 1024], bf, tag="pt",
                                        name=f"pt_{rep}_{h}_{kb}_{hs}")
                          nc.scalar.activation(pt[:, :hw], st[:, :hw], Exp,
                                               scale=0.125)
                          if causal and hs == 0:
                              nc.vector.tensor_mul(pt[:, :128], pt[:, :128],
                                                   tri_t[:, :])
                          pts.append(pt)
                      # QK for strip kb+1 goes to PE before PV of strip kb so
                      # the PE never stalls behind ACT
                      if kb + 1 < KB:
                          sts = emit_qk(kb + 1)
                      # PV accumulation for strip kb
                      if MODE != "full":
                          continue
                      for (q0, hs, hw, tg), pt in zip(strip_halves(kb), pts):
                          qs_range = list(range((q0 + hs) // 128,
                                                (q0 + hs + hw) // 128))
                          # the diagonal q_sub (== kb) additionally depends on
                          # the DVE tri-multiply; emit it last so the PE can
                          # start the other PV matmuls as soon as exp is done.
                          # (at kb==0 keep ascending order: the bank-group
                          # start=True matmuls must be first into each bank)
                          if causal and kb > 0 and qs_range and qs_range[0] == kb:
                              qs_range = qs_range[1:] + [kb]
                          for q_sub in qs_range:
                              m = q_sub * 128 - q0 - hs
                              last_kb = q_sub if causal else KB - 1
                              nc.tensor.matmul(
                                  acc(q_sub),
                                  lhsT=pt[:, m:m + 128],
                                  rhs=vg_t[:, h, kb, :],
                                  start=(kb == 0 and q_sub in _BANK_FIRST),
                                  stop=(q_sub in _BANK_LAST and kb == last_kb),
                              )
                      # normalize accumulator banks as soon as they complete
                      if causal:
                          if kb == 6:
                              normalize_bank(0, 7)
                          elif kb == 13:
                              normalize_bank(7, 14)
                          elif kb == 15:
                              normalize_bank(14, 16)
                  if not causal:
                      normalize_bank(0, 7)
                      normalize_bank(7, 14)
                      normalize_bank(14, 16)


def build_nc(causal=True, reps=1):
    """Build + compile the per-core Bass program (cached)."""
    key = ("nc", causal, reps, STRIP_W, MODE, ST_BUFS, PT_BUFS, STAGGERED,
           VERSION)
    if key in _built:
        return _built[key]
    import concourse.bacc as bacc
    from concourse import mybir, tile

    nc = bacc.Bacc("TRN2", target_bir_lowering=False, debug=False,
                   num_devices=N_CORES)
    qt = nc.dram_tensor("qt", (HEADS_PER_CORE // 2, 128, S),
                        mybir.dt.bfloat16, kind="ExternalInput").ap()
    kt = nc.dram_tensor("kt", (HEADS_PER_CORE // 2, 128, S),
                        mybir.dt.bfloat16, kind="ExternalInput").ap()
    vg = nc.dram_tensor("vg", (HEADS_PER_CORE, 128, KB, 65),
                        mybir.dt.bfloat16, kind="ExternalInput").ap()
    tri = nc.dram_tensor("tri", (128, 128), mybir.dt.bfloat16,
                         kind="ExternalInput").ap()
    o = nc.dram_tensor("o", (HEADS_PER_CORE, S, D), mybir.dt.float32,
                       kind="ExternalOutput").ap()
    with tile.TileContext(nc) as tc:
        if VERSION >= 5 and causal:
            _emit_v5(tc, nc, mybir, qt, kt, vg, tri, o, reps)
        elif VERSION >= 3 and causal:
            _emit_v3(tc, nc, mybir, qt, kt, vg, tri, o, reps)
        else:
            _emit(tc, nc, mybir, qt, kt, vg, tri, o, causal, reps)
    nc.compile()
    _built[key] = nc
    return nc


def prep_inputs(Q, K, V):
    """Host-side shard + layout prep. Returns list of 8 per-core input dicts."""
    Qf = np.ascontiguousarray(Q, dtype=np.float32).reshape(B * H, S, D)
    Kf = np.ascontiguousarray(K, dtype=np.float32).reshape(B * H, S, D)
    Vf = np.ascontiguousarray(V, dtype=np.float32).reshape(B * H, S, D)

    # [BH, S, D] -> transposed, bf16: [BH, D, S]
    Qt = np.ascontiguousarray(Qf.transpose(0, 2, 1)).astype(_BF16)
    Kt = np.ascontiguousarray(Kf.transpose(0, 2, 1)).astype(_BF16)

    # V augmented with ones column, partition-major: [BH, 128, KB, 65]
    Vb = Vf.astype(_BF16)
    vg_all = np.empty((B * H, 128, KB, 65), dtype=_BF16)
    # V[h, kb*128 + r, c] -> vg[h, r, kb, c]
    vg_all[:, :, :, :64] = Vb.reshape(B * H, KB, 128, D).transpose(0, 2, 1, 3)
    vg_all[:, :, :, 64] = _BF16(1.0)

    tri_np = (np.tril(np.ones((128, 128), dtype=np.float32))
              .T.astype(_BF16))  # tri[k, c] = 1 if c >= k
    tri_np = np.ascontiguousarray(tri_np)

    in_maps = []
    for c in range(N_CORES):
        h0 = c * HEADS_PER_CORE
        qt_c = np.empty((HEADS_PER_CORE // 2, 128, S), dtype=_BF16)
        kt_c = np.empty((HEADS_PER_CORE // 2, 128, S), dtype=_BF16)
        for p in range(HEADS_PER_CORE // 2):
            qt_c[p, :64] = Qt[h0 + 2 * p]
            qt_c[p, 64:] = Qt[h0 + 2 * p + 1]
            kt_c[p, :64] = Kt[h0 + 2 * p]
            kt_c[p, 64:] = Kt[h0 + 2 * p + 1]
        in_maps.append({
            "qt": qt_c,
            "kt": kt_c,
            "vg": np.ascontiguousarray(vg_all[h0:h0 + HEADS_PER_CORE]),
            "tri": tri_np,
        })
    return in_maps


def _classify_mask(mask):
    m = np.asarray(mask).reshape(S, S)
    if not m.any():
        return "dense"
    if np.array_equal(m, np.triu(np.ones((S, S), dtype=bool), k=1)):
        return "causal"
    raise NotImplementedError("only causal or all-False masks supported")


def run_cores(in_maps, causal=True, reps=1, **kwargs):
    from concourse import bass_utils

    nc = build_nc(causal, reps)
    return bass_utils.run_bass_kernel_spmd(
        nc, in_maps, core_ids=list(range(N_CORES)), **kwargs
    )


def kernel(Q, K, V, mask):
    kind = _classify_mask(mask)
    in_maps = prep_inputs(Q, K, V)
    res = run_cores(in_maps, causal=(kind == "causal"))
    out = np.concatenate([r["o"] for r in res.results], axis=0)
    return out.reshape(B, H, S, D).astype(np.float32)


if __name__ == "__main__":
    rng = np.random.default_rng(0)
    Q = rng.standard_normal((B, H, S, D), dtype=np.float32)
    K = rng.standard_normal((B, H, S, D), dtype=np.float32)
    V = rng.standard_normal((B, H, S, D), dtype=np.float32)
    mask = np.triu(np.ones((S, S), dtype=bool), k=1)[None, None]
    out = kernel(Q, K, V, mask)
    print("out", out.shape, out.dtype)

